# revision 1
# baseline (speedup 1.0000x reference)
"""Sliding-window GQA attention (T=4096, DIM=2048, H=16, KVH=4, D=128, W=1024)
as an 8-core SPMD Trainium2 Bass/Tile kernel.

Sharding: sequence-parallel. Core c owns queries [512c, 512c+512) and
recomputes K/V for its sliding window (1536 kv slots, zero-padded before
position 0). No collectives.

Dataflow (everything transposed so softmax needs no cross-partition max):
  Q^T[h] [d=128, q=512]   = RoPE(Wq_h^T x_q^T)        (per head)
  K^T[kvh] [128, 1536]    = RoPE(Wk_kvh^T x_kv^T)
  V[m] [t=128, 512=kvh*d] = x_kv[tile]^T^T ... natural layout per t-tile
  S^T [t-tile, q-span]    = K-tile(stationary) @ Q^T   (PSUM)
  P^T = exp(scale*S^T + kbias[t])   (ACT, fp32r out; kbias kills padded t)
  P^T *= triangle masks on boundary blocks (DVE)
  Y^T[h] += V-tile @ P^T ; den[h] += ones @ P^T        (PSUM accumulate)
  Y^T[h] = Y^T * (1/den)                               (softmax normalize)
  O^T[e-tile] += Wo-chunk(stationary) @ Y^T[h]         -> DRAM [2048, 512]

Host side: transposes, fp32r rounding (11-bit mantissa), RoPE tables with
sign-folded sin, masks, gather/unTranspose of per-core outputs.
"""

import math
import os
import sys

import numpy as np


def _ensure_paths():
    for p in (
        "/root/.axon_site",
        "/root/.axon_site/_ro/trn_rl_repo",
        "/root/.axon_site/_ro/pypackages",
        "/opt/trn_rl_repo",
        "/opt/pypackages",
    ):
        if os.path.isdir(p) and p not in sys.path:
            sys.path.append(p)


try:
    import concourse.bass as bass  # noqa: F401
except ImportError:
    _ensure_paths()

import concourse.bass as bass
import concourse.mybir as mybir
import concourse.tile as tile
from concourse import bacc
from concourse.bass_utils import run_bass_kernel_spmd

# ---------------------------------------------------------------- constants
N_CORES = 8
T = 4096
DIM = 2048
H = 16
KVH = 4
D = 128
WIN = 1024
ROPE_BASE = 10000.0

TQ = T // N_CORES          # 512 queries per core
TKV = TQ + WIN             # 1536 kv slots per core
NMT = TKV // 128           # 12 kv tiles of 128
NCC = DIM // 128           # 16 contraction chunks
SCALE = 1.0 / math.sqrt(D)
GQ = H // KVH              # 4 q heads per kv head

F32 = mybir.dt.float32
F32R = mybir.dt.float32r
BF16 = mybir.dt.bfloat16

# per kv-tile m: (qlo, qhi) span of local queries it can interact with
SPANS = {
    0: (0, 256), 1: (0, 256), 2: (0, 384), 3: (0, 512),
    4: (0, 512), 5: (0, 512), 6: (0, 512), 7: (0, 512),
    8: (0, 512), 9: (128, 512), 10: (256, 512), 11: (256, 512),
}
# per kv-tile m: (mask_name, local_lo, local_hi) or None
# per kv-tile m: (mask_name, lo, hi, zero_lo, zero_hi) in absolute q coords
MASKS = {
    0: ("maskB", 0, 128, 128, 256), 1: ("maskB", 128, 256, None, None),
    2: ("maskB", 256, 384, None, None), 3: ("maskB", 384, 512, None, None),
    4: None, 5: None, 6: None, 7: None,
    8: ("maskA", 0, 128, None, None), 9: ("maskA", 128, 256, None, None),
    10: ("maskA", 256, 384, None, None), 11: ("maskA", 384, 512, 256, 384),
}
# PSUM accumulation order: m=4 first (full-width span -> start=True clears
# the whole Y/den bank), m=11 last (stop=True).
M_ORDER = [4, 5, 6, 7, 0, 1, 2, 3, 8, 9, 10, 11]


def round_f32r(x):
    """fp32 -> fp32r: round-to-nearest-even to 11 mantissa bits."""
    b = np.ascontiguousarray(x, dtype=np.float32).view(np.uint32)
    b = (b + np.uint32(0x7FF) + ((b >> np.uint32(12)) & np.uint32(1))) & np.uint32(
        0xFFFFF000
    )
    return b.view(np.float32)


# ---------------------------------------------------------------- device code
_NC_CACHE = None


def _build():
    global _NC_CACHE
    if _NC_CACHE is not None:
        return _NC_CACHE

    nc = bacc.Bacc("TRN2", target_bir_lowering=False, debug=False,
                   num_devices=N_CORES)

    # DRAM I/O (per-core contents supplied via in_maps)
    xqT = nc.dram_tensor("xqT", [DIM, TQ], F32R, kind="ExternalInput").ap()
    xkvT = nc.dram_tensor("xkvT", [3 * DIM, 512], F32R, kind="ExternalInput").ap()
    wq = nc.dram_tensor("wq", [8 * DIM, 256], F32R, kind="ExternalInput").ap()
    wk = nc.dram_tensor("wk", [DIM, KVH * D], F32R, kind="ExternalInput").ap()
    wv = nc.dram_tensor("wv", [DIM, KVH * D], F32R, kind="ExternalInput").ap()
    wo = nc.dram_tensor("wo", [8 * DIM, 256], F32R, kind="ExternalInput").ap()
    cosq = nc.dram_tensor("cosq", [D, TQ], F32, kind="ExternalInput").ap()
    sinq = nc.dram_tensor("sinq", [D, TQ], F32, kind="ExternalInput").ap()
    cosk = nc.dram_tensor("cosk", [3 * D, 512], F32, kind="ExternalInput").ap()
    sink = nc.dram_tensor("sink", [3 * D, 512], F32, kind="ExternalInput").ap()
    kbias = nc.dram_tensor("kbias", [128, NMT], F32, kind="ExternalInput").ap()
    maskB = nc.dram_tensor("maskB", [128, 128], F32, kind="ExternalInput").ap()
    maskA = nc.dram_tensor("maskA", [128, 128], F32, kind="ExternalInput").ap()
    rotp = nc.dram_tensor("rotp", [128, 128], F32R, kind="ExternalInput").ap()
    ones = nc.dram_tensor("ones", [128, 128], F32R, kind="ExternalInput").ap()
    outT = nc.dram_tensor("outT", [DIM, TQ], F32, kind="ExternalOutput").ap()

    mask_dram = {"maskB": maskB, "maskA": maskA}

    with tile.TileContext(nc) as tc:
        _emit(nc, tc, xqT, xkvT, wq, wk, wv, wo, cosq, sinq, cosk, sink,
              kbias, mask_dram, rotp, ones, outT)

    nc.compile()
    _NC_CACHE = nc
    return nc


def _emit(nc, tc, xqT, xkvT, wq, wk, wv, wo, cosq, sinq, cosk, sink,
          kbias, mask_dram, rotp, ones, outT):
    from contextlib import ExitStack

    ctx = ExitStack()
    with ctx:
        # pools
        consts = ctx.enter_context(tc.tile_pool(name="consts", bufs=1))
        xbuf = ctx.enter_context(tc.tile_pool(name="xbuf", bufs=18))
        wqp = ctx.enter_context(tc.tile_pool(name="wqp", bufs=3))
        wres = ctx.enter_context(tc.tile_pool(name="wres", bufs=NCC))
        wvp = ctx.enter_context(tc.tile_pool(name="wvp", bufs=6))
        wop = ctx.enter_context(tc.tile_pool(name="wop", bufs=8))
        qtp = ctx.enter_context(tc.tile_pool(name="qtp", bufs=4))
        ktp = ctx.enter_context(tc.tile_pool(name="ktp", bufs=KVH))
        vp = ctx.enter_context(tc.tile_pool(name="vp", bufs=NMT))
        ytp = ctx.enter_context(tc.tile_pool(name="ytp", bufs=H))
        pp = ctx.enter_context(tc.tile_pool(name="pp", bufs=2))
        tmp = ctx.enter_context(tc.tile_pool(name="tmp", bufs=2))
        t12 = ctx.enter_context(tc.tile_pool(name="t12", bufs=3))
        fin = ctx.enter_context(tc.tile_pool(name="fin", bufs=2))
        ps_a = ctx.enter_context(tc.tile_pool(name="ps_a", bufs=2, space="PSUM"))
        ps_b = ctx.enter_context(tc.tile_pool(name="ps_b", bufs=2, space="PSUM"))
        ps_s = ctx.enter_context(tc.tile_pool(name="ps_s", bufs=2, space="PSUM"))
        ps_y = ctx.enter_context(tc.tile_pool(name="ps_y", bufs=2, space="PSUM"))

        Exp = mybir.ActivationFunctionType.Exp

        # ---- constants into SBUF
        def cload(ap, shape, dtype, tag):
            t = consts.tile(shape, dtype, tag=tag)
            nc.sync.dma_start(t[:], ap[:])
            return t

        rotp_sb = cload(rotp, [128, 128], F32R, "rotp")
        ones_sb = cload(ones, [128, 128], F32R, "ones")
        kbias_sb = cload(kbias, [128, NMT], F32, "kbias")
        cosq_sb = cload(cosq, [D, TQ], F32, "cosq")
        sinq_sb = cload(sinq, [D, TQ], F32, "sinq")
        mask_sb = {
            name: cload(mask_dram[name], [128, 128], F32, name)
            for name in ("maskB", "maskA")
        }

        def rope(src_ps, sin_sl, cos_sl, dst_ap, width):
            """dst = src*cos + rot_half(src)*sin  (dst fp32r)."""
            s_sb = tmp.tile([128, 512], F32R, tag="ropesb")
            nc.vector.tensor_copy(s_sb[:, :width], src_ps[:, :width])
            r_ps = ps_b.tile([128, 512], F32, tag="ps_b")
            nc.tensor.matmul(r_ps[:, :width], rotp_sb[:], s_sb[:, :width],
                             start=True, stop=True)
            t1 = t12.tile([128, 512], F32, tag="t12")
            nc.vector.tensor_mul(t1[:, :width], r_ps[:, :width], sin_sl)
            t2 = t12.tile([128, 512], F32, tag="t12")
            nc.vector.tensor_mul(t2[:, :width], src_ps[:, :width], cos_sl)
            nc.vector.tensor_add(dst_ap, t1[:, :width], t2[:, :width])

        # ---- phase A: K^T (RoPE'd) and V over 3 spans of 512 kv slots
        kt_sb = [ktp.tile([128, TKV], F32R, tag="kt", name=f"kt{g}")
                 for g in range(KVH)]
        v_sb = [vp.tile([128, 512], F32R, tag="v", name=f"v{m}")
                for m in range(NMT)]
        wk_res = []
        for c in range(NCC):
            wkt = wres.tile([128, 512], F32R, tag="wres", name=f"wkres{c}")
            nc.gpsimd.dma_start(wkt[:], wk[c * 128:(c + 1) * 128, :])
            wk_res.append(wkt)

        for s in range(3):
            xs = []
            for c in range(NCC):
                xt = xbuf.tile([128, 512], F32R, tag="xb")
                nc.sync.dma_start(
                    xt[:], xkvT[s * DIM + c * 128:s * DIM + (c + 1) * 128, :])
                xs.append(xt)
            cosk_s = xbuf.tile([128, 512], F32, tag="xb")
            nc.sync.dma_start(cosk_s[:], cosk[s * 128:(s + 1) * 128, :])
            sink_s = xbuf.tile([128, 512], F32, tag="xb")
            nc.sync.dma_start(sink_s[:], sink[s * 128:(s + 1) * 128, :])

            # K^T projection: c-outer across 4 psum banks (wk slab DMAs)
            kps = [ps_s.tile([128, 512], F32, tag="ps_s", name=f"kps{s}_0"),
                   ps_s.tile([128, 512], F32, tag="ps_s", name=f"kps{s}_1"),
                   ps_y.tile([128, 512], F32, tag="ps_y", name=f"kps{s}_2"),
                   ps_y.tile([128, 512], F32, tag="ps_y", name=f"kps{s}_3")]
            for c in range(NCC):
                for g in range(KVH):
                    nc.tensor.matmul(kps[g][:],
                                     wk_res[c][:, g * 128:(g + 1) * 128],
                                     xs[c][:],
                                     start=(c == 0), stop=(c == NCC - 1))
            for g in range(KVH):
                rope(kps[g], sink_s[:], cosk_s[:],
                     kt_sb[g][:, s * 512:(s + 1) * 512], 512)

            # V projection (natural layout): c-outer across 4 psum banks
            vps = [ps_a.tile([128, 512], F32, tag="ps_a", name=f"vps{s}_0"),
                   ps_a.tile([128, 512], F32, tag="ps_a", name=f"vps{s}_1"),
                   ps_b.tile([128, 512], F32, tag="ps_b", name=f"vps{s}_2"),
                   ps_b.tile([128, 512], F32, tag="ps_b", name=f"vps{s}_3")]
            for c in range(NCC):
                wvt = wvp.tile([128, 512], F32R, tag="wv")
                nc.sync.dma_start(wvt[:], wv[c * 128:(c + 1) * 128, :])
                for tt in range(4):
                    nc.tensor.matmul(
                        vps[tt][:],
                        xs[c][:, tt * 128:(tt + 1) * 128],
                        wvt[:],
                        start=(c == 0), stop=(c == NCC - 1))
            for tt in range(4):
                nc.vector.tensor_copy(v_sb[4 * s + tt][:], vps[tt][:])

        # ---- phases B+C interleaved per head
        xq_sb = []
        for c in range(NCC):
            xt = xbuf.tile([128, 512], F32R, tag="xb")
            nc.sync.dma_start(xt[:], xqT[c * 128:(c + 1) * 128, :])
            xq_sb.append(xt)

        yt_sb = [ytp.tile([128, TQ], F32R, tag="yt", name=f"yt{h}")
                 for h in range(H)]

        qts = {}

        def emit_pair_proj(p_):
            h0 = 2 * p_
            qpair = [ps_a.tile([128, 512], F32, tag="ps_a",
                               name=f"qps{h0}_{j}") for j in range(2)]
            for c in range(NCC):
                wqt = wqp.tile([128, 256], F32R, tag="wq",
                               name=f"wqt{h0}_{c}")
                nc.gpsimd.dma_start(
                    wqt[:],
                    wq[p_ * DIM + c * 128:p_ * DIM + (c + 1) * 128, :])
                for j in range(2):
                    nc.tensor.matmul(qpair[j][:],
                                     wqt[:, j * 128:(j + 1) * 128],
                                     xq_sb[c][:],
                                     start=(c == 0), stop=(c == NCC - 1))
            for j in range(2):
                qtj = qtp.tile([128, TQ], F32R, tag="qt", name=f"qt{h0}_{j}")
                rope(qpair[j], sinq_sb[:], cosq_sb[:], qtj[:], TQ)
                qts[h0 + j] = qtj

        def emit_attn(h):
            g = h // GQ
            qt = qts[h]
            yps = ps_y.tile([128, TQ], F32, tag="ps_y", name=f"yps{h}")
            dps = ps_b.tile([128, TQ], F32, tag="ps_b", name=f"dps{h}")
            for mi, m in enumerate(M_ORDER):
                qlo, qhi = SPANS[m]
                w = qhi - qlo
                sps = ps_s.tile([128, 512], F32, tag="ps_s", name=f"sps{h}_{m}")
                nc.tensor.matmul(sps[:, :w],
                                 kt_sb[g][:, m * 128:(m + 1) * 128],
                                 qt[:, qlo:qhi], start=True, stop=True)
                p = pp.tile([128, 512], F32R, tag="p", name=f"p{h}_{m}")
                nc.scalar.activation(p[:, :w], sps[:, :w], Exp,
                                     bias=kbias_sb[:, m:m + 1], scale=SCALE)
                mk = MASKS[m]
                if mk is not None:
                    name, lo, hi, zlo, zhi = mk
                    nc.vector.tensor_mul(p[:, lo - qlo:hi - qlo],
                                         p[:, lo - qlo:hi - qlo],
                                         mask_sb[name][:])
                    if zlo is not None:
                        nc.vector.tensor_scalar_mul(
                            p[:, zlo - qlo:zhi - qlo],
                            p[:, zlo - qlo:zhi - qlo], 0.0)
                first = mi == 0
                last = mi == len(M_ORDER) - 1
                nc.tensor.matmul(yps[:, qlo:qhi],
                                 v_sb[m][:, g * 128:(g + 1) * 128],
                                 p[:, :w], start=first, stop=last)
                nc.tensor.matmul(dps[:, qlo:qhi], ones_sb[:], p[:, :w],
                                 start=first, stop=last)

            rcp = fin.tile([128, TQ], F32, tag="rcp", name=f"rcp{h}")
            nc.vector.reciprocal(rcp[:], dps[:])
            nc.vector.tensor_mul(yt_sb[h][:], yps[:], rcp[:])

        # one-pair lookahead: emit projections a pair ahead of attention
        emit_pair_proj(0)
        for p_ in range(H // 2):
            if p_ + 1 < H // 2:
                emit_pair_proj(p_ + 1)
            emit_attn(2 * p_)
            emit_attn(2 * p_ + 1)

        # ---- phase D: O^T projection in e-tile pairs
        for n0 in range(0, NCC, 2):
            opair = [ps_a.tile([128, 512], F32, tag="ps_a",
                               name=f"ops{n0}_{j}") for j in range(2)]
            for h in range(H):
                wot = wop.tile([128, 256], F32R, tag="wo")
                np_ = n0 // 2
                nc.sync.dma_start(
                    wot[:],
                    wo[np_ * DIM + h * 128:np_ * DIM + (h + 1) * 128, :])
                for j in range(2):
                    nc.tensor.matmul(opair[j][:],
                                     wot[:, j * 128:(j + 1) * 128],
                                     yt_sb[h][:],
                                     start=(h == 0), stop=(h == H - 1))
            for j in range(2):
                osb = fin.tile([128, TQ], F32, tag="osb")
                nc.vector.tensor_copy(osb[:], opair[j][:])
                nc.sync.dma_start(outT[(n0 + j) * 128:(n0 + j + 1) * 128, :],
                                  osb[:])


# ---------------------------------------------------------------- host side
def _host_inputs(x, Wq, Wk, Wv, Wo):
    x = np.asarray(x, dtype=np.float32).reshape(T, DIM)

    inv_freq = 1.0 / (ROPE_BASE ** (np.arange(0, D, 2, dtype=np.float64) / D))
    dfreq = np.concatenate([inv_freq, inv_freq])  # [128] per-dim freq

    wq_r = round_f32r(
        np.asarray(Wq).reshape(DIM, 8, 256).transpose(1, 0, 2).reshape(8 * DIM, 256))
    wk_r = round_f32r(Wk)
    wv_r = round_f32r(Wv)
    wo_r = round_f32r(
        np.asarray(Wo).reshape(DIM, 8, 256).transpose(1, 0, 2).reshape(8 * DIM, 256))

    u = np.arange(128)[:, None]
    maskB = (np.arange(128)[None, :] < u).astype(np.float32)        # qq>=u -> 0
    maskA = (u <= np.arange(128)[None, :]).astype(np.float32)       # u>qq -> 0

    rotp = np.zeros((128, 128), np.float32)
    d = np.arange(128)
    rotp[(d + 64) % 128, d] = 1.0  # out[d] = in[(d+64)%128]

    ones = np.ones((128, 128), np.float32)

    in_maps = []
    for c in range(N_CORES):
        qs = c * TQ
        xq = x[qs:qs + TQ]                      # [512, 2048]
        xkv = np.zeros((TKV, DIM), np.float32)  # [1536, 2048]
        lo = qs - WIN
        src_lo = max(0, lo)
        xkv[src_lo - lo:TKV] = x[src_lo:qs + TQ]

        pos_q = np.arange(qs, qs + TQ, dtype=np.float64)
        pos_k = np.arange(lo, qs + TQ, dtype=np.float64)
        angq = dfreq[:, None] * pos_q[None, :]  # [128, 512]
        angk = dfreq[:, None] * pos_k[None, :]  # [128, 1536]
        sgn = np.where(np.arange(D) < D // 2, -1.0, 1.0)[:, None]

        kb = np.zeros((128, NMT), np.float32)
        for m in range(NMT):
            t_abs = 128 * m + np.arange(128)
            kb[:, m] = np.where(t_abs < WIN - qs, -30.0, 0.0)

        in_maps.append({
            "xqT": round_f32r(xq.T),
            "xkvT": round_f32r(
                xkv.T.reshape(DIM, 3, 512).transpose(1, 0, 2).reshape(3 * DIM, 512)),
            "wq": wq_r, "wk": wk_r, "wv": wv_r, "wo": wo_r,  # wq/wo pre-paired
            "cosq": np.cos(angq).astype(np.float32),
            "sinq": (sgn * np.sin(angq)).astype(np.float32),
            "cosk": np.ascontiguousarray(np.cos(angk).astype(np.float32)
                .reshape(D, 3, 512).transpose(1, 0, 2)).reshape(3 * D, 512),
            "sink": np.ascontiguousarray(((sgn * np.sin(angk)).astype(np.float32))
                .reshape(D, 3, 512).transpose(1, 0, 2)).reshape(3 * D, 512),
            "kbias": kb,
            "maskB": maskB, "maskA": maskA,
            "rotp": round_f32r(rotp),
            "ones": round_f32r(ones),
        })
    return in_maps


def kernel(x, Wq, Wk, Wv, Wo, _trace=False, _trace_kwargs=None):
    nc = _build()
    in_maps = _host_inputs(x, Wq, Wk, Wv, Wo)
    res = run_bass_kernel_spmd(nc, in_maps, core_ids=list(range(N_CORES)),
                               trace=_trace, **(_trace_kwargs or {}))
    out = np.empty((1, T, DIM), np.float32)
    for c in range(N_CORES):
        out[0, c * TQ:(c + 1) * TQ, :] = res.results[c]["outT"].T
    if _trace:
        kernel.last_results = res
    return out



# revision 2
# speedup vs baseline: 1.3824x; 1.3824x over previous
"""Sliding-window GQA attention (T=4096, DIM=2048, H=16, KVH=4, D=128, W=1024)
as an 8-core SPMD Trainium2 Bass/Tile kernel.

Sharding: sequence-parallel. Core c owns queries [512c, 512c+512) and
recomputes K/V for its sliding window (1536 kv slots, zero-padded before
position 0). No collectives.

v2: all matmul operands in bf16 (halves DMA + LDWEIGHTS, PSUM stays fp32),
exact attention spans (4608 cols/head vs 4864), four clean phases:
  A : K^T (RoPE'd, bf16) and V (natural, bf16) over 3 spans of 512 kv slots,
      rope matmuls interleaved into the V chains to keep PE dense
  A2: all 16 Q^T heads projected + RoPE'd up front
  B : attention, software-pipelined: S(m+2) issued before PV(m)/den(m) so
      PE never waits on exp (ACT) / mask (DVE); LOOK=2, 3 PSUM S-banks
  C : O^T projection; wo prefetched into SBUF during B via queue ordering
Softmax denominator reciprocal via the fast custom-DVE op; PSUM->SBUF
copies moved to the scalar (ACT) engine where it is otherwise idle.

Host side: transposes, bf16 casts, RoPE tables with sign-folded sin,
masks, gather/unTranspose of per-core outputs.
"""

import math
import os
import sys

import numpy as np


def _ensure_paths():
    for p in (
        "/root/.axon_site",
        "/root/.axon_site/_ro/trn_rl_repo",
        "/root/.axon_site/_ro/pypackages",
        "/opt/trn_rl_repo",
        "/opt/pypackages",
    ):
        if os.path.isdir(p) and p not in sys.path:
            sys.path.append(p)


try:
    import concourse.bass as bass  # noqa: F401
except ImportError:
    _ensure_paths()

import ml_dtypes

import concourse.bass as bass
import concourse.mybir as mybir
import concourse.tile as tile
from concourse import bacc
from concourse.bass_utils import run_bass_kernel_spmd

# ---------------------------------------------------------------- constants
N_CORES = 8
T = 4096
DIM = 2048
H = 16
KVH = 4
D = 128
WIN = 1024
ROPE_BASE = 10000.0

TQ = T // N_CORES          # 512 queries per core
TKV = TQ + WIN             # 1536 kv slots per core
NMT = TKV // 128           # 12 kv tiles of 128
NCC = DIM // 128           # 16 contraction chunks
SCALE = 1.0 / math.sqrt(D)
GQ = H // KVH              # 4 q heads per kv head

F32 = mybir.dt.float32
BF16 = mybir.dt.bfloat16
BF = ml_dtypes.bfloat16

# per kv-tile m: exact (qlo, qhi) span of local queries it can interact with
SPANS = {
    0: (0, 128), 1: (0, 256), 2: (0, 384), 3: (0, 512),
    4: (0, 512), 5: (0, 512), 6: (0, 512), 7: (0, 512),
    8: (0, 512), 9: (128, 512), 10: (256, 512), 11: (384, 512),
}
# per kv-tile m: (mask_name, lo, hi) triangle block in absolute q coords
MASKS = {
    0: ("maskB", 0, 128), 1: ("maskB", 128, 256),
    2: ("maskB", 256, 384), 3: ("maskB", 384, 512),
    4: None, 5: None, 6: None, 7: None,
    8: ("maskA", 0, 128), 9: ("maskA", 128, 256),
    10: ("maskA", 256, 384), 11: ("maskA", 384, 512),
}
# PSUM accumulation order: m=4 first (full-width span -> start=True clears
# the whole Y/den bank), m=11 last (stop=True).
M_ORDER = [4, 5, 6, 7, 0, 1, 2, 3, 8, 9, 10, 11]
LOOK = 2                   # S-matmul lookahead depth in phase B


# ---------------------------------------------------------------- device code
_NC_CACHE = None


def _build():
    global _NC_CACHE
    if _NC_CACHE is not None:
        return _NC_CACHE

    nc = bacc.Bacc("TRN2", target_bir_lowering=False, debug=False,
                   num_devices=N_CORES)

    # DRAM I/O (per-core contents supplied via in_maps)
    xqT = nc.dram_tensor("xqT", [DIM, TQ], BF16, kind="ExternalInput").ap()
    xkvT = nc.dram_tensor("xkvT", [3 * DIM, 512], BF16, kind="ExternalInput").ap()
    wq = nc.dram_tensor("wq", [8 * DIM, 256], BF16, kind="ExternalInput").ap()
    wk = nc.dram_tensor("wk", [DIM, KVH * D], BF16, kind="ExternalInput").ap()
    wv = nc.dram_tensor("wv", [DIM, KVH * D], BF16, kind="ExternalInput").ap()
    wo = nc.dram_tensor("wo", [8 * DIM, 256], BF16, kind="ExternalInput").ap()
    cosq = nc.dram_tensor("cosq", [D, TQ], F32, kind="ExternalInput").ap()
    sinq = nc.dram_tensor("sinq", [D, TQ], F32, kind="ExternalInput").ap()
    cosk = nc.dram_tensor("cosk", [3 * D, 512], F32, kind="ExternalInput").ap()
    sink = nc.dram_tensor("sink", [3 * D, 512], F32, kind="ExternalInput").ap()
    kbias = nc.dram_tensor("kbias", [128, NMT], F32, kind="ExternalInput").ap()
    maskB = nc.dram_tensor("maskB", [128, 128], BF16, kind="ExternalInput").ap()
    maskA = nc.dram_tensor("maskA", [128, 128], BF16, kind="ExternalInput").ap()
    rotp = nc.dram_tensor("rotp", [128, 128], BF16, kind="ExternalInput").ap()
    ones = nc.dram_tensor("ones", [128, 128], BF16, kind="ExternalInput").ap()
    outT = nc.dram_tensor("outT", [DIM, TQ], F32, kind="ExternalOutput").ap()

    mask_dram = {"maskB": maskB, "maskA": maskA}

    with tile.TileContext(nc) as tc:
        _emit(nc, tc, xqT, xkvT, wq, wk, wv, wo, cosq, sinq, cosk, sink,
              kbias, mask_dram, rotp, ones, outT)

    nc.compile()
    _NC_CACHE = nc
    return nc


def _emit(nc, tc, xqT, xkvT, wq, wk, wv, wo, cosq, sinq, cosk, sink,
          kbias, mask_dram, rotp, ones, outT):
    from contextlib import ExitStack

    ctx = ExitStack()
    with ctx:
        # SBUF pools (sizes are per-partition bytes; total ~196KB < 208KB)
        consts = ctx.enter_context(tc.tile_pool(name="consts", bufs=1))
        ropet = ctx.enter_context(tc.tile_pool(name="ropet", bufs=4))
        xsp = ctx.enter_context(tc.tile_pool(name="xsp", bufs=17))
        wkp = ctx.enter_context(tc.tile_pool(name="wkp", bufs=NCC))
        wvp = ctx.enter_context(tc.tile_pool(name="wvp", bufs=NCC))
        wqp = ctx.enter_context(tc.tile_pool(name="wqp", bufs=32))
        wop = ctx.enter_context(tc.tile_pool(name="wop", bufs=48))
        xqp = ctx.enter_context(tc.tile_pool(name="xqp", bufs=NCC))
        qtp = ctx.enter_context(tc.tile_pool(name="qtp", bufs=H))
        ktp = ctx.enter_context(tc.tile_pool(name="ktp", bufs=KVH))
        vp = ctx.enter_context(tc.tile_pool(name="vp", bufs=NMT))
        ytp = ctx.enter_context(tc.tile_pool(name="ytp", bufs=H))
        pp = ctx.enter_context(tc.tile_pool(name="pp", bufs=4))
        tmp = ctx.enter_context(tc.tile_pool(name="tmp", bufs=6))
        t12 = ctx.enter_context(tc.tile_pool(name="t12", bufs=4))
        fin = ctx.enter_context(tc.tile_pool(name="fin", bufs=2))
        # PSUM: exactly 8 banks
        ps_acc = ctx.enter_context(tc.tile_pool(name="ps_acc", bufs=4, space="PSUM"))
        ps_s = ctx.enter_context(tc.tile_pool(name="ps_s", bufs=3, space="PSUM"))
        ps_r = ctx.enter_context(tc.tile_pool(name="ps_r", bufs=1, space="PSUM"))

        Exp = mybir.ActivationFunctionType.Exp
        Copy = mybir.ActivationFunctionType.Copy

        # ---- persistent weights (gpsimd DMA queue)
        wk_res = []
        for c in range(NCC):
            wkt = wkp.tile([128, 512], BF16, tag="wk", name=f"wkres{c}")
            nc.gpsimd.dma_start(wkt[:], wk[c * 128:(c + 1) * 128, :])
            wk_res.append(wkt)
        wv_res = []
        for c in range(NCC):
            wvt = wvp.tile([128, 512], BF16, tag="wv", name=f"wvres{c}")
            nc.gpsimd.dma_start(wvt[:], wv[c * 128:(c + 1) * 128, :])
            wv_res.append(wvt)

        # ---- phase A: K^T (RoPE'd) and V over 3 spans of 512 kv slots
        kt_sb = [ktp.tile([128, TKV], BF16, tag="kt", name=f"kt{g}")
                 for g in range(KVH)]
        v_sb = [vp.tile([128, 512], BF16, tag="v", name=f"v{m}")
                for m in range(NMT)]

        consts_loaded = [False]
        const_sb = {}

        def cload(ap, shape, dtype, tag):
            t = consts.tile(shape, dtype, tag=tag, name=tag)
            nc.sync.dma_start(t[:], ap[:])
            return t

        for s in range(3):
            xs = []
            for c in range(NCC):
                xt = xsp.tile([128, 512], BF16, tag="xs", name=f"xs{s}_{c}")
                nc.sync.dma_start(
                    xt[:], xkvT[s * DIM + c * 128:s * DIM + (c + 1) * 128, :])
                xs.append(xt)
            cosk_s = ropet.tile([128, 512], F32, tag="rt", name=f"cosk{s}")
            nc.sync.dma_start(cosk_s[:], cosk[s * 128:(s + 1) * 128, :])
            sink_s = ropet.tile([128, 512], F32, tag="rt", name=f"sink{s}")
            nc.sync.dma_start(sink_s[:], sink[s * 128:(s + 1) * 128, :])
            if not consts_loaded[0]:
                const_sb["rotp"] = cload(rotp, [128, 128], BF16, "rotp")
                const_sb["ones"] = cload(ones, [128, 128], BF16, "ones")
                const_sb["kbias"] = cload(kbias, [128, NMT], F32, "kbias")
                const_sb["maskB"] = cload(mask_dram["maskB"], [128, 128],
                                          BF16, "maskB")
                const_sb["maskA"] = cload(mask_dram["maskA"], [128, 128],
                                          BF16, "maskA")
                consts_loaded[0] = True

            # K^T projection: 4 chains (one per kv head) across acc banks
            kps = [ps_acc.tile([128, 512], F32, tag="acc", name=f"kps{s}_{g}")
                   for g in range(KVH)]
            for c in range(NCC):
                for g in range(KVH):
                    nc.tensor.matmul(kps[g][:],
                                     wk_res[c][:, g * 128:(g + 1) * 128],
                                     xs[c][:],
                                     start=(c == 0), stop=(c == NCC - 1))
            # rope rotate-half sources, copied early on ACT
            ssb = []
            for g in range(KVH):
                sg = tmp.tile([128, 512], BF16, tag="ssb", name=f"ssb{s}_{g}")
                nc.scalar.activation(sg[:], kps[g][:], Copy)
                ssb.append(sg)

            def ropek(g):
                r_ps = ps_r.tile([128, 512], F32, tag="rp", name=f"rk{s}_{g}")
                nc.tensor.matmul(r_ps[:], const_sb["rotp"][:], ssb[g][:],
                                 start=True, stop=True)
                t1 = t12.tile([128, 512], F32, tag="t12", name=f"kt1_{s}_{g}")
                nc.vector.tensor_mul(t1[:], r_ps[:], sink_s[:])
                t2 = t12.tile([128, 512], F32, tag="t12", name=f"kt2_{s}_{g}")
                nc.vector.tensor_mul(t2[:], kps[g][:], cosk_s[:])
                nc.vector.tensor_add(kt_sb[g][:, s * 512:(s + 1) * 512],
                                     t1[:], t2[:])

            # V projection (natural layout) in 2 passes of 2 PSUM banks,
            # with the 4 rope matmuls interleaved between V-chain batches
            vps = {}
            def vchain(tts, c0, c1):
                for c in range(c0, c1):
                    for tt in tts:
                        nc.tensor.matmul(
                            vps[tt][:],
                            xs[c][:, tt * 128:(tt + 1) * 128],
                            wv_res[c][:],
                            start=(c == 0), stop=(c == NCC - 1))

            for tt in (0, 1):
                vps[tt] = ps_s.tile([128, 512], F32, tag="sps",
                                    name=f"vps{s}_{tt}")
            vchain((0, 1), 0, 8)
            ropek(0)
            vchain((0, 1), 8, NCC)
            ropek(1)
            for tt in (0, 1):
                nc.scalar.activation(v_sb[4 * s + tt][:], vps[tt][:], Copy)
            for tt in (2, 3):
                vps[tt] = ps_s.tile([128, 512], F32, tag="sps",
                                    name=f"vps{s}_{tt}")
            vchain((2, 3), 0, 8)
            ropek(2)
            vchain((2, 3), 8, NCC)
            ropek(3)
            for tt in (2, 3):
                nc.scalar.activation(v_sb[4 * s + tt][:], vps[tt][:], Copy)

        # ---- phase A2: all 16 Q^T heads projected + RoPE'd
        xq_sb = []
        for c in range(NCC):
            xt = xqp.tile([128, 512], BF16, tag="xq", name=f"xq{c}")
            nc.sync.dma_start(xt[:], xqT[c * 128:(c + 1) * 128, :])
            xq_sb.append(xt)
        cosq_sb = ropet.tile([128, 512], F32, tag="rt", name="cosq")
        nc.sync.dma_start(cosq_sb[:], cosq[:])
        sinq_sb = ropet.tile([128, 512], F32, tag="rt", name="sinq")
        nc.sync.dma_start(sinq_sb[:], sinq[:])

        qts = {}
        for p_ in range(H // 2):
            qpair = [ps_acc.tile([128, 512], F32, tag="acc",
                                 name=f"qps{p_}_{j}") for j in range(2)]
            for c in range(NCC):
                wqt = wqp.tile([128, 256], BF16, tag="wq",
                               name=f"wqt{p_}_{c}")
                nc.gpsimd.dma_start(
                    wqt[:],
                    wq[p_ * DIM + c * 128:p_ * DIM + (c + 1) * 128, :])
                for j in range(2):
                    nc.tensor.matmul(qpair[j][:],
                                     wqt[:, j * 128:(j + 1) * 128],
                                     xq_sb[c][:],
                                     start=(c == 0), stop=(c == NCC - 1))
            for j in range(2):
                sg = tmp.tile([128, 512], BF16, tag="ssb", name=f"sq{p_}_{j}")
                nc.scalar.activation(sg[:], qpair[j][:], Copy)
                r_ps = ps_s.tile([128, 512], F32, tag="sps",
                                 name=f"rq{p_}_{j}")
                nc.tensor.matmul(r_ps[:], const_sb["rotp"][:], sg[:],
                                 start=True, stop=True)
                t1 = t12.tile([128, 512], F32, tag="t12", name=f"qt1_{p_}_{j}")
                nc.vector.tensor_mul(t1[:], r_ps[:], sinq_sb[:])
                t2 = t12.tile([128, 512], F32, tag="t12", name=f"qt2_{p_}_{j}")
                nc.vector.tensor_mul(t2[:], qpair[j][:], cosq_sb[:])
                qtj = qtp.tile([128, 512], BF16, tag="qt", name=f"qt{2*p_+j}")
                nc.vector.tensor_add(qtj[:], t1[:], t2[:])
                qts[2 * p_ + j] = qtj

        # ---- phase B: attention, software-pipelined per head
        yt_sb = [ytp.tile([128, TQ], BF16, tag="yt", name=f"yt{h}")
                 for h in range(H)]

        for h in range(H):
            g = h // GQ
            qt = qts[h]
            acc_y = ps_acc.tile([128, TQ], F32, tag="acc", name=f"yps{h}")
            acc_d = ps_acc.tile([128, TQ], F32, tag="acc", name=f"dps{h}")
            p_l = {}

            def qk(mi, h=h, g=g, qt=qt, p_l=p_l):
                m = M_ORDER[mi]
                qlo, qhi = SPANS[m]
                w = qhi - qlo
                sps = ps_s.tile([128, 512], F32, tag="sps",
                                name=f"sps{h}_{m}")
                nc.tensor.matmul(sps[:, :w],
                                 kt_sb[g][:, m * 128:(m + 1) * 128],
                                 qt[:, qlo:qhi], start=True, stop=True)
                p = pp.tile([128, 512], BF16, tag="p", name=f"p{h}_{m}")
                nc.scalar.activation(p[:, :w], sps[:, :w], Exp,
                                     bias=const_sb["kbias"][:, m:m + 1],
                                     scale=SCALE)
                mk = MASKS[m]
                if mk is not None:
                    name_, lo, hi = mk
                    nc.vector.tensor_mul(p[:, lo - qlo:hi - qlo],
                                         p[:, lo - qlo:hi - qlo],
                                         const_sb[name_][:])
                p_l[mi] = p

            def pv(mi, h=h, g=g, acc_y=acc_y, acc_d=acc_d, p_l=p_l):
                m = M_ORDER[mi]
                qlo, qhi = SPANS[m]
                w = qhi - qlo
                p = p_l.pop(mi)
                first = mi == 0
                last = mi == NMT - 1
                nc.tensor.matmul(acc_y[:, qlo:qhi],
                                 v_sb[m][:, g * 128:(g + 1) * 128],
                                 p[:, :w], start=first, stop=last)
                nc.tensor.matmul(acc_d[:, qlo:qhi], const_sb["ones"][:],
                                 p[:, :w], start=first, stop=last)

            for i in range(LOOK):
                qk(i)
            for i in range(NMT):
                if i + LOOK < NMT:
                    qk(i + LOOK)
                pv(i)

            rcp = fin.tile([128, TQ], F32, tag="rcp", name=f"rcp{h}")
            nc.vector.reciprocal_approx_fast(rcp[:], acc_d[:])
            nc.vector.tensor_mul(yt_sb[h][:], acc_y[:], rcp[:])

        # ---- phase C: O^T projection in e-tile pairs (wo prefetched by
        # sync-queue ordering + wop pool depth during phase B)
        for n0 in range(0, NCC, 2):
            opair = [ps_acc.tile([128, 512], F32, tag="acc",
                                 name=f"ops{n0}_{j}") for j in range(2)]
            np_ = n0 // 2
            for h in range(H):
                wot = wop.tile([128, 256], BF16, tag="wo",
                               name=f"wot{n0}_{h}")
                nc.sync.dma_start(
                    wot[:],
                    wo[np_ * DIM + h * 128:np_ * DIM + (h + 1) * 128, :])
                for j in range(2):
                    nc.tensor.matmul(opair[j][:],
                                     wot[:, j * 128:(j + 1) * 128],
                                     yt_sb[h][:],
                                     start=(h == 0), stop=(h == H - 1))
            for j in range(2):
                osb = fin.tile([128, TQ], F32, tag="osb", name=f"osb{n0}_{j}")
                nc.scalar.activation(osb[:], opair[j][:], Copy)
                nc.sync.dma_start(outT[(n0 + j) * 128:(n0 + j + 1) * 128, :],
                                  osb[:])


# ---------------------------------------------------------------- host side
def _host_inputs(x, Wq, Wk, Wv, Wo):
    x = np.asarray(x, dtype=np.float32).reshape(T, DIM)

    inv_freq = 1.0 / (ROPE_BASE ** (np.arange(0, D, 2, dtype=np.float64) / D))
    dfreq = np.concatenate([inv_freq, inv_freq])  # [128] per-dim freq

    wq_r = np.asarray(Wq).reshape(DIM, 8, 256).transpose(1, 0, 2) \
        .reshape(8 * DIM, 256).astype(BF)
    wk_r = np.asarray(Wk, np.float32).astype(BF)
    wv_r = np.asarray(Wv, np.float32).astype(BF)
    wo_r = np.asarray(Wo).reshape(DIM, 8, 256).transpose(1, 0, 2) \
        .reshape(8 * DIM, 256).astype(BF)

    u = np.arange(128)[:, None]
    maskB = (np.arange(128)[None, :] < u).astype(BF)        # qq>=u -> 0
    maskA = (u <= np.arange(128)[None, :]).astype(BF)       # u>qq -> 0

    rotp = np.zeros((128, 128), np.float32)
    d = np.arange(128)
    rotp[(d + 64) % 128, d] = 1.0  # out[d] = in[(d+64)%128]
    rotp = rotp.astype(BF)

    ones = np.ones((128, 128), BF)

    in_maps = []
    for c in range(N_CORES):
        qs = c * TQ
        xq = x[qs:qs + TQ]                      # [512, 2048]
        xkv = np.zeros((TKV, DIM), np.float32)  # [1536, 2048]
        lo = qs - WIN
        src_lo = max(0, lo)
        xkv[src_lo - lo:TKV] = x[src_lo:qs + TQ]

        pos_q = np.arange(qs, qs + TQ, dtype=np.float64)
        pos_k = np.arange(lo, qs + TQ, dtype=np.float64)
        angq = dfreq[:, None] * pos_q[None, :]  # [128, 512]
        angk = dfreq[:, None] * pos_k[None, :]  # [128, 1536]
        sgn = np.where(np.arange(D) < D // 2, -1.0, 1.0)[:, None]

        kb = np.zeros((128, NMT), np.float32)
        for m in range(NMT):
            t_abs = 128 * m + np.arange(128)
            kb[:, m] = np.where(t_abs < WIN - qs, -30.0, 0.0)

        in_maps.append({
            "xqT": np.ascontiguousarray(xq.T).astype(BF),
            "xkvT": np.ascontiguousarray(
                xkv.T.reshape(DIM, 3, 512).transpose(1, 0, 2)
                .reshape(3 * DIM, 512)).astype(BF),
            "wq": wq_r, "wk": wk_r, "wv": wv_r, "wo": wo_r,  # wq/wo pre-paired
            "cosq": np.cos(angq).astype(np.float32),
            "sinq": (sgn * np.sin(angq)).astype(np.float32),
            "cosk": np.ascontiguousarray(np.cos(angk).astype(np.float32)
                .reshape(D, 3, 512).transpose(1, 0, 2)).reshape(3 * D, 512),
            "sink": np.ascontiguousarray(((sgn * np.sin(angk)).astype(np.float32))
                .reshape(D, 3, 512).transpose(1, 0, 2)).reshape(3 * D, 512),
            "kbias": kb,
            "maskB": maskB, "maskA": maskA,
            "rotp": rotp,
            "ones": ones,
        })
    return in_maps


def kernel(x, Wq, Wk, Wv, Wo, _trace=False, _trace_kwargs=None):
    nc = _build()
    in_maps = _host_inputs(x, Wq, Wk, Wv, Wo)
    res = run_bass_kernel_spmd(nc, in_maps, core_ids=list(range(N_CORES)),
                               trace=_trace, **(_trace_kwargs or {}))
    out = np.empty((1, T, DIM), np.float32)
    for c in range(N_CORES):
        out[0, c * TQ:(c + 1) * TQ, :] = res.results[c]["outT"].T
    if _trace:
        kernel.last_results = res
    return out


# revision 3
# speedup vs baseline: 1.4990x; 1.0844x over previous
"""Sliding-window GQA attention (T=4096, DIM=2048, H=16, KVH=4, D=128, W=1024)
as an 8-core SPMD Trainium2 Bass/Tile kernel.

Sharding: sequence-parallel. Core c owns queries [512c, 512c+512) and
recomputes K/V for its sliding window (1536 kv slots, zero-padded before
position 0). No collectives.

v3: all matmul operands bf16 (PSUM fp32), exact attention spans, four
phases, fat-row DMA layouts (weights packed so each DMA moves 8-16KB per
partition line instead of 512B descriptors), software-pipelined RoPE
(rope matmuls of pair p emitted mid-chain of pair p+1), deferred g=3
K-chain start to hide cross-span PSUM WAR waits:
  A : K^T (RoPE'd, bf16) and V (natural, bf16) over 3 spans of 512 kv slots
  A2: all 16 Q^T heads projected + RoPE'd
  B : attention, S(m+2) issued before PV(m)/den(m); LOOK=2
  C : O^T projection, wo streamed as 8KB-row pair tiles
Softmax denominator reciprocal via fast custom-DVE op; PSUM->SBUF copies
on the scalar (ACT) engine.
"""

import math
import os
import sys

import numpy as np


def _ensure_paths():
    for p in (
        "/root/.axon_site",
        "/root/.axon_site/_ro/trn_rl_repo",
        "/root/.axon_site/_ro/pypackages",
        "/opt/trn_rl_repo",
        "/opt/pypackages",
    ):
        if os.path.isdir(p) and p not in sys.path:
            sys.path.append(p)


try:
    import concourse.bass as bass  # noqa: F401
except ImportError:
    _ensure_paths()

import ml_dtypes

import concourse.bass as bass
import concourse.mybir as mybir
import concourse.tile as tile
from concourse import bacc
from concourse.bass_utils import run_bass_kernel_spmd

# ---------------------------------------------------------------- constants
N_CORES = 8
T = 4096
DIM = 2048
H = 16
KVH = 4
D = 128
WIN = 1024
ROPE_BASE = 10000.0

TQ = T // N_CORES          # 512 queries per core
TKV = TQ + WIN             # 1536 kv slots per core
NMT = TKV // 128           # 12 kv tiles of 128
NCC = DIM // 128           # 16 contraction chunks
SCALE = 1.0 / math.sqrt(D)
GQ = H // KVH              # 4 q heads per kv head

F32 = mybir.dt.float32
BF16 = mybir.dt.bfloat16
BF = ml_dtypes.bfloat16

# per kv-tile m: exact (qlo, qhi) span of local queries it can interact with
SPANS = {
    0: (0, 128), 1: (0, 256), 2: (0, 384), 3: (0, 512),
    4: (0, 512), 5: (0, 512), 6: (0, 512), 7: (0, 512),
    8: (0, 512), 9: (128, 512), 10: (256, 512), 11: (384, 512),
}
# per kv-tile m: (mask_name, lo, hi) triangle block in absolute q coords
MASKS = {
    0: ("maskB", 0, 128), 1: ("maskB", 128, 256),
    2: ("maskB", 256, 384), 3: ("maskB", 384, 512),
    4: None, 5: None, 6: None, 7: None,
    8: ("maskA", 0, 128), 9: ("maskA", 128, 256),
    10: ("maskA", 256, 384), 11: ("maskA", 384, 512),
}
# PSUM accumulation order: m=4 first (full-width span -> start=True clears
# the whole Y/den bank), m=11 last (stop=True).
M_ORDER = [4, 5, 6, 7, 0, 1, 2, 3, 8, 9, 10, 11]
LOOK = 2                   # S-matmul lookahead depth in phase B


# ---------------------------------------------------------------- device code
_NC_CACHE = None


def _build():
    global _NC_CACHE
    if _NC_CACHE is not None:
        return _NC_CACHE

    nc = bacc.Bacc("TRN2", target_bir_lowering=False, debug=False,
                   num_devices=N_CORES)

    # DRAM I/O (per-core contents supplied via in_maps). Weight layouts are
    # packed so every DMA moves a fat contiguous row per partition:
    #   wq[p*128+r, c*256+jc]   = Wq[c*128+r, p*256+jc]     (8KB rows)
    #   wo[np*128+r, h*256+jc]  = Wo[h*128+r, np*256+jc]    (8KB rows)
    #   wk[r, c*512+e]          = Wk[c*128+r, e]            (16KB rows)
    #   wv[r, c*512+e]          = Wv[c*128+r, e]            (16KB rows)
    #   xq[r, c*512+q]          = x[qs+q, c*128+r]          (16KB rows)
    xq = nc.dram_tensor("xq", [128, NCC * TQ], BF16, kind="ExternalInput").ap()
    xkvT = nc.dram_tensor("xkvT", [3 * DIM, 512], BF16, kind="ExternalInput").ap()
    wq = nc.dram_tensor("wq", [8 * 128, 4096], BF16, kind="ExternalInput").ap()
    wk = nc.dram_tensor("wk", [128, NCC * 512], BF16, kind="ExternalInput").ap()
    wv = nc.dram_tensor("wv", [128, NCC * 512], BF16, kind="ExternalInput").ap()
    wo = nc.dram_tensor("wo", [8 * 128, 4096], BF16, kind="ExternalInput").ap()
    cosq = nc.dram_tensor("cosq", [D, TQ], F32, kind="ExternalInput").ap()
    sinq = nc.dram_tensor("sinq", [D, TQ], F32, kind="ExternalInput").ap()
    cosk = nc.dram_tensor("cosk", [3 * D, 512], F32, kind="ExternalInput").ap()
    sink = nc.dram_tensor("sink", [3 * D, 512], F32, kind="ExternalInput").ap()
    kbias = nc.dram_tensor("kbias", [128, NMT], F32, kind="ExternalInput").ap()
    maskB = nc.dram_tensor("maskB", [128, 128], BF16, kind="ExternalInput").ap()
    maskA = nc.dram_tensor("maskA", [128, 128], BF16, kind="ExternalInput").ap()
    rotp = nc.dram_tensor("rotp", [128, 128], BF16, kind="ExternalInput").ap()
    ones = nc.dram_tensor("ones", [128, 128], BF16, kind="ExternalInput").ap()
    outT = nc.dram_tensor("outT", [DIM, TQ], F32, kind="ExternalOutput").ap()

    mask_dram = {"maskB": maskB, "maskA": maskA}

    with tile.TileContext(nc) as tc:
        _emit(nc, tc, xq, xkvT, wq, wk, wv, wo, cosq, sinq, cosk, sink,
              kbias, mask_dram, rotp, ones, outT)

    nc.compile()
    _NC_CACHE = nc
    return nc


def _emit(nc, tc, xq, xkvT, wq, wk, wv, wo, cosq, sinq, cosk, sink,
          kbias, mask_dram, rotp, ones, outT):
    from contextlib import ExitStack

    ctx = ExitStack()
    with ctx:
        # SBUF pools (sizes are per-partition bytes; total ~202KB < 208KB)
        consts = ctx.enter_context(tc.tile_pool(name="consts", bufs=1))
        ropet = ctx.enter_context(tc.tile_pool(name="ropet", bufs=4))
        xsp = ctx.enter_context(tc.tile_pool(name="xsp", bufs=17))
        wkp = ctx.enter_context(tc.tile_pool(name="wkp", bufs=1))
        wvp = ctx.enter_context(tc.tile_pool(name="wvp", bufs=1))
        wqp = ctx.enter_context(tc.tile_pool(name="wqp", bufs=3))
        wop = ctx.enter_context(tc.tile_pool(name="wop", bufs=3))
        xqp = ctx.enter_context(tc.tile_pool(name="xqp", bufs=1))
        qtp = ctx.enter_context(tc.tile_pool(name="qtp", bufs=H))
        ktp = ctx.enter_context(tc.tile_pool(name="ktp", bufs=KVH))
        vp = ctx.enter_context(tc.tile_pool(name="vp", bufs=NMT))
        ytp = ctx.enter_context(tc.tile_pool(name="ytp", bufs=H))
        pp = ctx.enter_context(tc.tile_pool(name="pp", bufs=4))
        tmp = ctx.enter_context(tc.tile_pool(name="tmp", bufs=4))
        t12 = ctx.enter_context(tc.tile_pool(name="t12", bufs=4))
        fin = ctx.enter_context(tc.tile_pool(name="fin", bufs=2))
        # PSUM: exactly 8 banks
        ps_acc = ctx.enter_context(tc.tile_pool(name="ps_acc", bufs=4, space="PSUM"))
        ps_s = ctx.enter_context(tc.tile_pool(name="ps_s", bufs=3, space="PSUM"))
        ps_r = ctx.enter_context(tc.tile_pool(name="ps_r", bufs=1, space="PSUM"))

        Exp = mybir.ActivationFunctionType.Exp
        Copy = mybir.ActivationFunctionType.Copy

        # ---- persistent weights (gpsimd DMA queue), single fat tiles
        wk_all = wkp.tile([128, NCC * 512], BF16, tag="wk", name="wk_all")
        nc.gpsimd.dma_start(wk_all[:], wk[:])
        wv_all = wvp.tile([128, NCC * 512], BF16, tag="wv", name="wv_all")
        nc.gpsimd.dma_start(wv_all[:], wv[:])

        # ---- phase A: K^T (RoPE'd) and V over 3 spans of 512 kv slots
        kt_sb = [ktp.tile([128, TKV], BF16, tag="kt", name=f"kt{g}")
                 for g in range(KVH)]
        v_sb = [vp.tile([128, 512], BF16, tag="v", name=f"v{m}")
                for m in range(NMT)]

        consts_loaded = [False]
        const_sb = {}

        def cload(ap, shape, dtype, tag):
            t = consts.tile(shape, dtype, tag=tag, name=tag)
            nc.sync.dma_start(t[:], ap[:])
            return t

        for s in range(3):
            xs = []
            for c in range(NCC):
                xt = xsp.tile([128, 512], BF16, tag="xs", name=f"xs{s}_{c}")
                nc.sync.dma_start(
                    xt[:], xkvT[s * DIM + c * 128:s * DIM + (c + 1) * 128, :])
                xs.append(xt)
            cosk_s = ropet.tile([128, 512], F32, tag="rt", name=f"cosk{s}")
            nc.sync.dma_start(cosk_s[:], cosk[s * 128:(s + 1) * 128, :])
            sink_s = ropet.tile([128, 512], F32, tag="rt", name=f"sink{s}")
            nc.sync.dma_start(sink_s[:], sink[s * 128:(s + 1) * 128, :])
            if not consts_loaded[0]:
                const_sb["rotp"] = cload(rotp, [128, 128], BF16, "rotp")
                const_sb["ones"] = cload(ones, [128, 128], BF16, "ones")
                const_sb["kbias"] = cload(kbias, [128, NMT], F32, "kbias")
                const_sb["maskB"] = cload(mask_dram["maskB"], [128, 128],
                                          BF16, "maskB")
                const_sb["maskA"] = cload(mask_dram["maskA"], [128, 128],
                                          BF16, "maskA")
                consts_loaded[0] = True

            # K^T projection: 4 chains (one per kv head) across acc banks.
            # g=3's first writes are deferred 12 matmuls so the WAR on last
            # span's kps[3] (read late by its rope t2-mul) is hidden.
            kps = [ps_acc.tile([128, 512], F32, tag="acc", name=f"kps{s}_{g}")
                   for g in range(KVH)]

            def kmm(c, g):
                nc.tensor.matmul(kps[g][:],
                                 wk_all[:, c * 512 + g * 128:
                                        c * 512 + (g + 1) * 128],
                                 xs[c][:],
                                 start=(c == 0), stop=(c == NCC - 1))

            for c in range(4):
                for g in range(3):
                    kmm(c, g)
            for c in range(4):
                kmm(c, 3)
            for c in range(4, NCC):
                for g in range(KVH):
                    kmm(c, g)

            # rope rotate-half sources, copied early on ACT
            ssb = []
            for g in range(KVH):
                sg = tmp.tile([128, 512], BF16, tag="ssb", name=f"ssb{s}_{g}")
                nc.scalar.activation(sg[:], kps[g][:], Copy)
                ssb.append(sg)

            def ropek(g):
                r_ps = ps_r.tile([128, 512], F32, tag="rp", name=f"rk{s}_{g}")
                nc.tensor.matmul(r_ps[:], const_sb["rotp"][:], ssb[g][:],
                                 start=True, stop=True)
                t1 = t12.tile([128, 512], F32, tag="t12", name=f"kt1_{s}_{g}")
                nc.vector.tensor_mul(t1[:], r_ps[:], sink_s[:])
                t2 = t12.tile([128, 512], F32, tag="t12", name=f"kt2_{s}_{g}")
                nc.vector.tensor_mul(t2[:], kps[g][:], cosk_s[:])
                nc.vector.tensor_add(kt_sb[g][:, s * 512:(s + 1) * 512],
                                     t1[:], t2[:])

            # V projection (natural layout) in 2 passes of 2 PSUM banks,
            # with the 4 rope matmuls interleaved between V-chain batches
            vps = {}

            def vchain(tts, c0, c1):
                for c in range(c0, c1):
                    for tt in tts:
                        nc.tensor.matmul(
                            vps[tt][:],
                            xs[c][:, tt * 128:(tt + 1) * 128],
                            wv_all[:, c * 512:(c + 1) * 512],
                            start=(c == 0), stop=(c == NCC - 1))

            for tt in (0, 1):
                vps[tt] = ps_s.tile([128, 512], F32, tag="sps",
                                    name=f"vps{s}_{tt}")
            vchain((0, 1), 0, 8)
            ropek(0)
            vchain((0, 1), 8, NCC)
            ropek(1)
            for tt in (0, 1):
                nc.scalar.activation(v_sb[4 * s + tt][:], vps[tt][:], Copy)
            for tt in (2, 3):
                vps[tt] = ps_s.tile([128, 512], F32, tag="sps",
                                    name=f"vps{s}_{tt}")
            vchain((2, 3), 0, 4)
            ropek(2)
            vchain((2, 3), 4, 8)
            ropek(3)
            vchain((2, 3), 8, NCC)
            for tt in (2, 3):
                nc.scalar.activation(v_sb[4 * s + tt][:], vps[tt][:], Copy)

        # ---- phase A2: all 16 Q^T heads projected + RoPE'd.
        # Rope matmuls of pair p are emitted mid-chain of pair p+1 so the
        # PE never waits on the ACT rotate-source copy.
        xq_all = xqp.tile([128, NCC * 512], BF16, tag="xq", name="xq_all")
        nc.sync.dma_start(xq_all[:], xq[:])
        cosq_sb = ropet.tile([128, 512], F32, tag="rt", name="cosq")
        nc.sync.dma_start(cosq_sb[:], cosq[:])
        sinq_sb = ropet.tile([128, 512], F32, tag="rt", name="sinq")
        nc.sync.dma_start(sinq_sb[:], sinq[:])

        qts = {}

        def ropeq(p_, j, qpair):
            sg = tmp.tile([128, 512], BF16, tag="ssb", name=f"sq{p_}_{j}")
            nc.scalar.activation(sg[:], qpair[j][:], Copy)
            r_ps = ps_s.tile([128, 512], F32, tag="sps", name=f"rq{p_}_{j}")
            nc.tensor.matmul(r_ps[:], const_sb["rotp"][:], sg[:],
                             start=True, stop=True)
            t1 = t12.tile([128, 512], F32, tag="t12", name=f"qt1_{p_}_{j}")
            nc.vector.tensor_mul(t1[:], r_ps[:], sinq_sb[:])
            t2 = t12.tile([128, 512], F32, tag="t12", name=f"qt2_{p_}_{j}")
            nc.vector.tensor_mul(t2[:], qpair[j][:], cosq_sb[:])
            qtj = qtp.tile([128, 512], BF16, tag="qt", name=f"qt{2 * p_ + j}")
            nc.vector.tensor_add(qtj[:], t1[:], t2[:])
            qts[2 * p_ + j] = qtj

        prev = None  # (p_, qpair) whose ropes are pending
        for p_ in range(H // 2):
            qpair = [ps_acc.tile([128, 512], F32, tag="acc",
                                 name=f"qps{p_}_{j}") for j in range(2)]
            wqt = wqp.tile([128, 4096], BF16, tag="wq", name=f"wqt{p_}")
            nc.gpsimd.dma_start(wqt[:], wq[p_ * 128:(p_ + 1) * 128, :])
            for c in range(NCC):
                if c == 6 and prev is not None:
                    ropeq(prev[0], 0, prev[1])
                if c == 10 and prev is not None:
                    ropeq(prev[0], 1, prev[1])
                    prev = None
                for j in range(2):
                    nc.tensor.matmul(qpair[j][:],
                                     wqt[:, c * 256 + j * 128:
                                         c * 256 + (j + 1) * 128],
                                     xq_all[:, c * 512:(c + 1) * 512],
                                     start=(c == 0), stop=(c == NCC - 1))
            prev = (p_, qpair)
        ropeq(prev[0], 0, prev[1])
        ropeq(prev[0], 1, prev[1])

        # ---- phase B: attention, software-pipelined per head
        yt_sb = [ytp.tile([128, TQ], BF16, tag="yt", name=f"yt{h}")
                 for h in range(H)]

        for h in range(H):
            g = h // GQ
            qt = qts[h]
            acc_y = ps_acc.tile([128, TQ], F32, tag="acc", name=f"yps{h}")
            acc_d = ps_acc.tile([128, TQ], F32, tag="acc", name=f"dps{h}")
            p_l = {}

            def qk(mi, h=h, g=g, qt=qt, p_l=p_l):
                m = M_ORDER[mi]
                qlo, qhi = SPANS[m]
                w = qhi - qlo
                sps = ps_s.tile([128, 512], F32, tag="sps",
                                name=f"sps{h}_{m}")
                nc.tensor.matmul(sps[:, :w],
                                 kt_sb[g][:, m * 128:(m + 1) * 128],
                                 qt[:, qlo:qhi], start=True, stop=True)
                p = pp.tile([128, 512], BF16, tag="p", name=f"p{h}_{m}")
                nc.scalar.activation(p[:, :w], sps[:, :w], Exp,
                                     bias=const_sb["kbias"][:, m:m + 1],
                                     scale=SCALE)
                mk = MASKS[m]
                if mk is not None:
                    name_, lo, hi = mk
                    nc.vector.tensor_mul(p[:, lo - qlo:hi - qlo],
                                         p[:, lo - qlo:hi - qlo],
                                         const_sb[name_][:])
                p_l[mi] = p

            def pv(mi, h=h, g=g, acc_y=acc_y, acc_d=acc_d, p_l=p_l):
                m = M_ORDER[mi]
                qlo, qhi = SPANS[m]
                w = qhi - qlo
                p = p_l.pop(mi)
                first = mi == 0
                last = mi == NMT - 1
                nc.tensor.matmul(acc_y[:, qlo:qhi],
                                 v_sb[m][:, g * 128:(g + 1) * 128],
                                 p[:, :w], start=first, stop=last)
                nc.tensor.matmul(acc_d[:, qlo:qhi], const_sb["ones"][:],
                                 p[:, :w], start=first, stop=last)

            for i in range(LOOK):
                qk(i)
            for i in range(NMT):
                if i + LOOK < NMT:
                    qk(i + LOOK)
                pv(i)

            rcp = fin.tile([128, TQ], F32, tag="rcp", name=f"rcp{h}")
            nc.vector.reciprocal_approx_fast(rcp[:], acc_d[:])
            nc.vector.tensor_mul(yt_sb[h][:], acc_y[:], rcp[:])

        # ---- phase C: O^T projection in e-tile pairs; wo streamed as
        # 8KB-row pair tiles (prefetch depth = wop bufs via queue ordering)
        for n0 in range(0, NCC, 2):
            np_ = n0 // 2
            wot = wop.tile([128, 4096], BF16, tag="wo", name=f"wot{np_}")
            nc.sync.dma_start(wot[:], wo[np_ * 128:(np_ + 1) * 128, :])
            opair = [ps_acc.tile([128, 512], F32, tag="acc",
                                 name=f"ops{n0}_{j}") for j in range(2)]
            for h in range(H):
                for j in range(2):
                    nc.tensor.matmul(opair[j][:],
                                     wot[:, h * 256 + j * 128:
                                         h * 256 + (j + 1) * 128],
                                     yt_sb[h][:],
                                     start=(h == 0), stop=(h == H - 1))
            for j in range(2):
                osb = fin.tile([128, TQ], F32, tag="osb", name=f"osb{n0}_{j}")
                nc.scalar.activation(osb[:], opair[j][:], Copy)
                nc.sync.dma_start(outT[(n0 + j) * 128:(n0 + j + 1) * 128, :],
                                  osb[:])


# ---------------------------------------------------------------- host side
def _host_inputs(x, Wq, Wk, Wv, Wo):
    x = np.asarray(x, dtype=np.float32).reshape(T, DIM)

    inv_freq = 1.0 / (ROPE_BASE ** (np.arange(0, D, 2, dtype=np.float64) / D))
    dfreq = np.concatenate([inv_freq, inv_freq])  # [128] per-dim freq

    # fat-row packed weight layouts (see _build comments)
    wq_r = np.ascontiguousarray(
        np.asarray(Wq).reshape(NCC, 128, 8, 256).transpose(2, 1, 0, 3)
        .reshape(8 * 128, 4096)).astype(BF)
    wo_r = np.ascontiguousarray(
        np.asarray(Wo).reshape(H, 128, 8, 256).transpose(2, 1, 0, 3)
        .reshape(8 * 128, 4096)).astype(BF)
    wk_r = np.ascontiguousarray(
        np.asarray(Wk, np.float32).reshape(NCC, 128, 512).transpose(1, 0, 2)
        .reshape(128, NCC * 512)).astype(BF)
    wv_r = np.ascontiguousarray(
        np.asarray(Wv, np.float32).reshape(NCC, 128, 512).transpose(1, 0, 2)
        .reshape(128, NCC * 512)).astype(BF)

    u = np.arange(128)[:, None]
    maskB = (np.arange(128)[None, :] < u).astype(BF)        # qq>=u -> 0
    maskA = (u <= np.arange(128)[None, :]).astype(BF)       # u>qq -> 0

    rotp = np.zeros((128, 128), np.float32)
    d = np.arange(128)
    rotp[(d + 64) % 128, d] = 1.0  # out[d] = in[(d+64)%128]
    rotp = rotp.astype(BF)

    ones = np.ones((128, 128), BF)

    in_maps = []
    for c in range(N_CORES):
        qs = c * TQ
        xqc = x[qs:qs + TQ]                     # [512, 2048]
        xkv = np.zeros((TKV, DIM), np.float32)  # [1536, 2048]
        lo = qs - WIN
        src_lo = max(0, lo)
        xkv[src_lo - lo:TKV] = x[src_lo:qs + TQ]

        pos_q = np.arange(qs, qs + TQ, dtype=np.float64)
        pos_k = np.arange(lo, qs + TQ, dtype=np.float64)
        angq = dfreq[:, None] * pos_q[None, :]  # [128, 512]
        angk = dfreq[:, None] * pos_k[None, :]  # [128, 1536]
        sgn = np.where(np.arange(D) < D // 2, -1.0, 1.0)[:, None]

        kb = np.zeros((128, NMT), np.float32)
        for m in range(NMT):
            t_abs = 128 * m + np.arange(128)
            kb[:, m] = np.where(t_abs < WIN - qs, -30.0, 0.0)

        in_maps.append({
            "xq": np.ascontiguousarray(
                xqc.T.reshape(NCC, 128, TQ).transpose(1, 0, 2)
                .reshape(128, NCC * TQ)).astype(BF),
            "xkvT": np.ascontiguousarray(
                xkv.T.reshape(DIM, 3, 512).transpose(1, 0, 2)
                .reshape(3 * DIM, 512)).astype(BF),
            "wq": wq_r, "wk": wk_r, "wv": wv_r, "wo": wo_r,
            "cosq": np.cos(angq).astype(np.float32),
            "sinq": (sgn * np.sin(angq)).astype(np.float32),
            "cosk": np.ascontiguousarray(np.cos(angk).astype(np.float32)
                .reshape(D, 3, 512).transpose(1, 0, 2)).reshape(3 * D, 512),
            "sink": np.ascontiguousarray(((sgn * np.sin(angk)).astype(np.float32))
                .reshape(D, 3, 512).transpose(1, 0, 2)).reshape(3 * D, 512),
            "kbias": kb,
            "maskB": maskB, "maskA": maskA,
            "rotp": rotp,
            "ones": ones,
        })
    return in_maps


def kernel(x, Wq, Wk, Wv, Wo, _trace=False, _trace_kwargs=None):
    nc = _build()
    in_maps = _host_inputs(x, Wq, Wk, Wv, Wo)
    res = run_bass_kernel_spmd(nc, in_maps, core_ids=list(range(N_CORES)),
                               trace=_trace, **(_trace_kwargs or {}))
    out = np.empty((1, T, DIM), np.float32)
    for c in range(N_CORES):
        out[0, c * TQ:(c + 1) * TQ, :] = res.results[c]["outT"].T
    if _trace:
        kernel.last_results = res
    return out


# revision 7
# speedup vs baseline: 1.5248x; 1.0172x over previous
"""Sliding-window GQA attention (T=4096, DIM=2048, H=16, KVH=4, D=128, W=1024)
as an 8-core SPMD Trainium2 Bass/Tile kernel.

Sharding: sequence-parallel. Core c owns queries [512c, 512c+512) and
recomputes K/V for its sliding window (1536 kv slots, zero-padded before
position 0). No collectives.

v3: all matmul operands bf16 (PSUM fp32), exact attention spans, four
phases, fat-row DMA layouts (weights packed so each DMA moves 8-16KB per
partition line instead of 512B descriptors), software-pipelined RoPE
(rope matmuls of pair p emitted mid-chain of pair p+1), deferred g=3
K-chain start to hide cross-span PSUM WAR waits:
  A : K^T (RoPE'd, bf16) and V (natural, bf16) over 3 spans of 512 kv slots
  A2: all 16 Q^T heads projected + RoPE'd
  B : attention, S(m+2) issued before PV(m)/den(m); LOOK=2
  C : O^T projection, wo streamed as 8KB-row pair tiles
Softmax denominator reciprocal via fast custom-DVE op; PSUM->SBUF copies
on the scalar (ACT) engine.
"""

import math
import os
import sys

import numpy as np


def _ensure_paths():
    for p in (
        "/root/.axon_site",
        "/root/.axon_site/_ro/trn_rl_repo",
        "/root/.axon_site/_ro/pypackages",
        "/opt/trn_rl_repo",
        "/opt/pypackages",
    ):
        if os.path.isdir(p) and p not in sys.path:
            sys.path.append(p)


try:
    import concourse.bass as bass  # noqa: F401
except ImportError:
    _ensure_paths()

import ml_dtypes

import concourse.bass as bass
import concourse.mybir as mybir
import concourse.tile as tile
from concourse import bacc
from concourse.bass_utils import run_bass_kernel_spmd

# ---------------------------------------------------------------- constants
N_CORES = 8
T = 4096
DIM = 2048
H = 16
KVH = 4
D = 128
WIN = 1024
ROPE_BASE = 10000.0

TQ = T // N_CORES          # 512 queries per core
TKV = TQ + WIN             # 1536 kv slots per core
NMT = TKV // 128           # 12 kv tiles of 128
NCC = DIM // 128           # 16 contraction chunks
SCALE = 1.0 / math.sqrt(D)
GQ = H // KVH              # 4 q heads per kv head

F32 = mybir.dt.float32
BF16 = mybir.dt.bfloat16
BF = ml_dtypes.bfloat16

# per kv-tile m: exact (qlo, qhi) span of local queries it can interact with
SPANS = {
    0: (0, 128), 1: (0, 256), 2: (0, 384), 3: (0, 512),
    4: (0, 512), 5: (0, 512), 6: (0, 512), 7: (0, 512),
    8: (0, 512), 9: (128, 512), 10: (256, 512), 11: (384, 512),
}
# per kv-tile m: (mask_name, lo, hi) triangle block in absolute q coords
MASKS = {
    0: ("maskB", 0, 128), 1: ("maskB", 128, 256),
    2: ("maskB", 256, 384), 3: ("maskB", 384, 512),
    4: None, 5: None, 6: None, 7: None,
    8: ("maskA", 0, 128), 9: ("maskA", 128, 256),
    10: ("maskA", 256, 384), 11: ("maskA", 384, 512),
}
# PSUM accumulation order: m=4 first (full-width span -> start=True clears
# the whole Y/den bank), m=11 last (stop=True).
M_ORDER = [4, 5, 6, 7, 0, 1, 2, 3, 8, 9, 10, 11]
LOOK = 2                   # S-matmul lookahead depth in phase B


# ---------------------------------------------------------------- device code
_NC_CACHE = None


def _build():
    global _NC_CACHE
    if _NC_CACHE is not None:
        return _NC_CACHE

    nc = bacc.Bacc("TRN2", target_bir_lowering=False, debug=False,
                   num_devices=N_CORES)

    # DRAM I/O (per-core contents supplied via in_maps). Weight layouts are
    # packed so every DMA moves a fat contiguous row per partition:
    #   wq[p*128+r, c*256+jc]   = Wq[c*128+r, p*256+jc]     (8KB rows)
    #   wo[np*128+r, h*256+jc]  = Wo[h*128+r, np*256+jc]    (8KB rows)
    #   wk[r, c*512+e]          = Wk[c*128+r, e]            (16KB rows)
    #   wv[r, c*512+e]          = Wv[c*128+r, e]            (16KB rows)
    #   xq[r, c*512+q]          = x[qs+q, c*128+r]          (16KB rows)
    xq = nc.dram_tensor("xq", [128, NCC * TQ], BF16, kind="ExternalInput").ap()
    xkvT = nc.dram_tensor("xkvT", [3 * DIM, 512], BF16, kind="ExternalInput").ap()
    wq = nc.dram_tensor("wq", [8 * 128, 4096], BF16, kind="ExternalInput").ap()
    wk = nc.dram_tensor("wk", [128, NCC * 512], BF16, kind="ExternalInput").ap()
    wv = nc.dram_tensor("wv", [128, NCC * 512], BF16, kind="ExternalInput").ap()
    wo = nc.dram_tensor("wo", [8 * 128, 4096], BF16, kind="ExternalInput").ap()
    cosq = nc.dram_tensor("cosq", [D, TQ], F32, kind="ExternalInput").ap()
    sinq = nc.dram_tensor("sinq", [D, TQ], F32, kind="ExternalInput").ap()
    cosk = nc.dram_tensor("cosk", [3 * D, 512], F32, kind="ExternalInput").ap()
    sink = nc.dram_tensor("sink", [3 * D, 512], F32, kind="ExternalInput").ap()
    kbias = nc.dram_tensor("kbias", [128, NMT], F32, kind="ExternalInput").ap()
    maskB = nc.dram_tensor("maskB", [128, 128], BF16, kind="ExternalInput").ap()
    maskA = nc.dram_tensor("maskA", [128, 128], BF16, kind="ExternalInput").ap()
    rotp = nc.dram_tensor("rotp", [128, 128], BF16, kind="ExternalInput").ap()
    ones = nc.dram_tensor("ones", [128, 128], BF16, kind="ExternalInput").ap()
    outT = nc.dram_tensor("outT", [DIM, TQ], F32, kind="ExternalOutput").ap()

    mask_dram = {"maskB": maskB, "maskA": maskA}

    with tile.TileContext(nc) as tc:
        _emit(nc, tc, xq, xkvT, wq, wk, wv, wo, cosq, sinq, cosk, sink,
              kbias, mask_dram, rotp, ones, outT)

    nc.compile()
    _NC_CACHE = nc
    return nc


def _emit(nc, tc, xq, xkvT, wq, wk, wv, wo, cosq, sinq, cosk, sink,
          kbias, mask_dram, rotp, ones, outT):
    from contextlib import ExitStack

    ctx = ExitStack()
    with ctx:
        # SBUF pools (sizes are per-partition bytes; total ~202KB < 208KB)
        consts = ctx.enter_context(tc.tile_pool(name="consts", bufs=1))
        ropet = ctx.enter_context(tc.tile_pool(name="ropet", bufs=4))
        xsp = ctx.enter_context(tc.tile_pool(name="xsp", bufs=17))
        wkp = ctx.enter_context(tc.tile_pool(name="wkp", bufs=1))
        wvp = ctx.enter_context(tc.tile_pool(name="wvp", bufs=1))
        wqp = ctx.enter_context(tc.tile_pool(name="wqp", bufs=3))
        wop = ctx.enter_context(tc.tile_pool(name="wop", bufs=3))
        xqp = ctx.enter_context(tc.tile_pool(name="xqp", bufs=1))
        qtp = ctx.enter_context(tc.tile_pool(name="qtp", bufs=H))
        ktp = ctx.enter_context(tc.tile_pool(name="ktp", bufs=KVH))
        vp = ctx.enter_context(tc.tile_pool(name="vp", bufs=NMT))
        ytp = ctx.enter_context(tc.tile_pool(name="ytp", bufs=H))
        pp = ctx.enter_context(tc.tile_pool(name="pp", bufs=4))
        tmp = ctx.enter_context(tc.tile_pool(name="tmp", bufs=4))
        t12 = ctx.enter_context(tc.tile_pool(name="t12", bufs=4))
        fin = ctx.enter_context(tc.tile_pool(name="fin", bufs=2))
        # PSUM: exactly 8 banks
        ps_acc = ctx.enter_context(tc.tile_pool(name="ps_acc", bufs=4, space="PSUM"))
        ps_s = ctx.enter_context(tc.tile_pool(name="ps_s", bufs=3, space="PSUM"))
        ps_r = ctx.enter_context(tc.tile_pool(name="ps_r", bufs=1, space="PSUM"))

        Exp = mybir.ActivationFunctionType.Exp
        Copy = mybir.ActivationFunctionType.Copy

        # ---- persistent weights (gpsimd DMA queue). Split into 4 sub-tiles
        # each so the first K/V chains start after ~512KB instead of 2MB,
        # and early xs tiles on the sync queue are not starved.
        wk_sub = []
        wv_sub = []
        for q4 in range(4):
            wkt = wkp.tile([128, 4 * 512], BF16, tag="wk", name=f"wk_sub{q4}",
                           bufs=4)
            nc.gpsimd.dma_start(wkt[:], wk[:, q4 * 2048:(q4 + 1) * 2048])
            wk_sub.append(wkt)
        for q4 in range(4):
            wvt = wvp.tile([128, 4 * 512], BF16, tag="wv", name=f"wv_sub{q4}",
                           bufs=4)
            nc.gpsimd.dma_start(wvt[:], wv[:, q4 * 2048:(q4 + 1) * 2048])
            wv_sub.append(wvt)

        def wk_sl(c, g):
            return wk_sub[c // 4][:, (c % 4) * 512 + g * 128:
                                  (c % 4) * 512 + (g + 1) * 128]

        def wv_sl(c):
            return wv_sub[c // 4][:, (c % 4) * 512:(c % 4 + 1) * 512]

        # ---- phase A: K^T (RoPE'd) and V over 3 spans of 512 kv slots
        kt_sb = [ktp.tile([128, TKV], BF16, tag="kt", name=f"kt{g}")
                 for g in range(KVH)]
        v_sb = [vp.tile([128, 512], BF16, tag="v", name=f"v{m}")
                for m in range(NMT)]

        consts_loaded = [False]
        const_sb = {}

        def cload(ap, shape, dtype, tag):
            t = consts.tile(shape, dtype, tag=tag, name=tag)
            nc.sync.dma_start(t[:], ap[:])
            return t

        for s in range(3):
            xs = []
            for c in range(NCC):
                xt = xsp.tile([128, 512], BF16, tag="xs", name=f"xs{s}_{c}")
                nc.sync.dma_start(
                    xt[:], xkvT[s * DIM + c * 128:s * DIM + (c + 1) * 128, :])
                xs.append(xt)
            cosk_s = ropet.tile([128, 512], F32, tag="rt", name=f"cosk{s}")
            nc.sync.dma_start(cosk_s[:], cosk[s * 128:(s + 1) * 128, :])
            sink_s = ropet.tile([128, 512], F32, tag="rt", name=f"sink{s}")
            nc.sync.dma_start(sink_s[:], sink[s * 128:(s + 1) * 128, :])
            if not consts_loaded[0]:
                const_sb["rotp"] = cload(rotp, [128, 128], BF16, "rotp")
                const_sb["ones"] = cload(ones, [128, 128], BF16, "ones")
                const_sb["kbias"] = cload(kbias, [128, NMT], F32, "kbias")
                const_sb["maskB"] = cload(mask_dram["maskB"], [128, 128],
                                          BF16, "maskB")
                const_sb["maskA"] = cload(mask_dram["maskA"], [128, 128],
                                          BF16, "maskA")
                consts_loaded[0] = True

            # K^T projection: 4 chains (one per kv head) across acc banks.
            # g=3's first writes are deferred 12 matmuls so the WAR on last
            # span's kps[3] (read late by its rope t2-mul) is hidden.
            kps = [ps_acc.tile([128, 512], F32, tag="acc", name=f"kps{s}_{g}")
                   for g in range(KVH)]

            def kmm(c, g):
                nc.tensor.matmul(kps[g][:], wk_sl(c, g), xs[c][:],
                                 start=(c == 0), stop=(c == NCC - 1))

            for c in range(4):
                for g in range(3):
                    kmm(c, g)
            for c in range(4):
                kmm(c, 3)
            for c in range(4, NCC):
                for g in range(KVH):
                    kmm(c, g)

            # rope rotate-half sources, copied early on ACT
            ssb = []
            for g in range(KVH):
                sg = tmp.tile([128, 512], BF16, tag="ssb", name=f"ssb{s}_{g}")
                nc.scalar.activation(sg[:], kps[g][:], Copy)
                ssb.append(sg)

            def ropek(g):
                r_ps = ps_r.tile([128, 512], F32, tag="rp", name=f"rk{s}_{g}")
                nc.tensor.matmul(r_ps[:], const_sb["rotp"][:], ssb[g][:],
                                 start=True, stop=True)
                t1 = t12.tile([128, 512], F32, tag="t12", name=f"kt1_{s}_{g}")
                nc.vector.tensor_mul(t1[:], r_ps[:], sink_s[:])
                t2 = t12.tile([128, 512], F32, tag="t12", name=f"kt2_{s}_{g}")
                nc.vector.tensor_mul(t2[:], kps[g][:], cosk_s[:])
                nc.vector.tensor_add(kt_sb[g][:, s * 512:(s + 1) * 512],
                                     t1[:], t2[:])

            # V projection (natural layout) in 2 passes of 2 PSUM banks,
            # with the 4 rope matmuls interleaved between V-chain batches
            vps = {}

            def vchain(tts, c0, c1):
                for c in range(c0, c1):
                    for tt in tts:
                        nc.tensor.matmul(
                            vps[tt][:],
                            xs[c][:, tt * 128:(tt + 1) * 128],
                            wv_sl(c),
                            start=(c == 0), stop=(c == NCC - 1))

            for tt in (0, 1):
                vps[tt] = ps_s.tile([128, 512], F32, tag="sps",
                                    name=f"vps{s}_{tt}")
            vchain((0, 1), 0, 8)
            ropek(0)
            vchain((0, 1), 8, NCC)
            ropek(1)
            for tt in (0, 1):
                nc.scalar.activation(v_sb[4 * s + tt][:], vps[tt][:], Copy)
            for tt in (2, 3):
                vps[tt] = ps_s.tile([128, 512], F32, tag="sps",
                                    name=f"vps{s}_{tt}")
            vchain((2, 3), 0, 4)
            ropek(2)
            vchain((2, 3), 4, 8)
            ropek(3)
            vchain((2, 3), 8, NCC)
            for tt in (2, 3):
                nc.scalar.activation(v_sb[4 * s + tt][:], vps[tt][:], Copy)

        # ---- phase A2: all 16 Q^T heads projected + RoPE'd.
        # Rope matmuls of pair p are emitted mid-chain of pair p+1 so the
        # PE never waits on the ACT rotate-source copy.
        xq_all = xqp.tile([128, NCC * 512], BF16, tag="xq", name="xq_all")
        nc.sync.dma_start(xq_all[:], xq[:])
        cosq_sb = ropet.tile([128, 512], F32, tag="rt", name="cosq")
        nc.sync.dma_start(cosq_sb[:], cosq[:])
        sinq_sb = ropet.tile([128, 512], F32, tag="rt", name="sinq")
        nc.sync.dma_start(sinq_sb[:], sinq[:])

        qts = {}

        def ropeq(p_, j, qpair):
            sg = tmp.tile([128, 512], BF16, tag="ssb", name=f"sq{p_}_{j}")
            nc.scalar.activation(sg[:], qpair[j][:], Copy)
            r_ps = ps_s.tile([128, 512], F32, tag="sps", name=f"rq{p_}_{j}")
            nc.tensor.matmul(r_ps[:], const_sb["rotp"][:], sg[:],
                             start=True, stop=True)
            t1 = t12.tile([128, 512], F32, tag="t12", name=f"qt1_{p_}_{j}")
            nc.vector.tensor_mul(t1[:], r_ps[:], sinq_sb[:])
            t2 = t12.tile([128, 512], F32, tag="t12", name=f"qt2_{p_}_{j}")
            nc.vector.tensor_mul(t2[:], qpair[j][:], cosq_sb[:])
            qtj = qtp.tile([128, 512], BF16, tag="qt", name=f"qt{2 * p_ + j}")
            nc.vector.tensor_add(qtj[:], t1[:], t2[:])
            qts[2 * p_ + j] = qtj

        prev = None  # (p_, qpair) whose ropes are pending
        for p_ in range(H // 2):
            qpair = [ps_acc.tile([128, 512], F32, tag="acc",
                                 name=f"qps{p_}_{j}") for j in range(2)]
            # sync queue: sits behind all xs tiles, so these 1MB transfers
            # cannot starve phase A's time-critical loads
            wqt = wqp.tile([128, 4096], BF16, tag="wq", name=f"wqt{p_}")
            nc.sync.dma_start(wqt[:], wq[p_ * 128:(p_ + 1) * 128, :])
            for c in range(NCC):
                if c == 6 and prev is not None:
                    ropeq(prev[0], 0, prev[1])
                if c == 10 and prev is not None:
                    ropeq(prev[0], 1, prev[1])
                    prev = None
                for j in range(2):
                    nc.tensor.matmul(qpair[j][:],
                                     wqt[:, c * 256 + j * 128:
                                         c * 256 + (j + 1) * 128],
                                     xq_all[:, c * 512:(c + 1) * 512],
                                     start=(c == 0), stop=(c == NCC - 1))
            prev = (p_, qpair)
        ropeq(prev[0], 0, prev[1])
        ropeq(prev[0], 1, prev[1])

        # ---- phase B: attention, software-pipelined per head
        yt_sb = [ytp.tile([128, TQ], BF16, tag="yt", name=f"yt{h}")
                 for h in range(H)]

        for h in range(H):
            g = h // GQ
            qt = qts[h]
            acc_y = ps_acc.tile([128, TQ], F32, tag="acc", name=f"yps{h}")
            acc_d = ps_acc.tile([128, TQ], F32, tag="acc", name=f"dps{h}")
            p_l = {}

            def qk(mi, h=h, g=g, qt=qt, p_l=p_l):
                m = M_ORDER[mi]
                qlo, qhi = SPANS[m]
                w = qhi - qlo
                sps = ps_s.tile([128, 512], F32, tag="sps",
                                name=f"sps{h}_{m}")
                nc.tensor.matmul(sps[:, :w],
                                 kt_sb[g][:, m * 128:(m + 1) * 128],
                                 qt[:, qlo:qhi], start=True, stop=True)
                p = pp.tile([128, 512], BF16, tag="p", name=f"p{h}_{m}")
                nc.scalar.activation(p[:, :w], sps[:, :w], Exp,
                                     bias=const_sb["kbias"][:, m:m + 1],
                                     scale=SCALE)
                mk = MASKS[m]
                if mk is not None:
                    name_, lo, hi = mk
                    nc.vector.tensor_mul(p[:, lo - qlo:hi - qlo],
                                         p[:, lo - qlo:hi - qlo],
                                         const_sb[name_][:])
                p_l[mi] = p

            def pv(mi, h=h, g=g, acc_y=acc_y, acc_d=acc_d, p_l=p_l):
                m = M_ORDER[mi]
                qlo, qhi = SPANS[m]
                w = qhi - qlo
                p = p_l.pop(mi)
                first = mi == 0
                last = mi == NMT - 1
                nc.tensor.matmul(acc_y[:, qlo:qhi],
                                 v_sb[m][:, g * 128:(g + 1) * 128],
                                 p[:, :w], start=first, stop=last)
                nc.tensor.matmul(acc_d[:, qlo:qhi], const_sb["ones"][:],
                                 p[:, :w], start=first, stop=last)

            for i in range(LOOK):
                qk(i)
            for i in range(NMT):
                if i + LOOK < NMT:
                    qk(i + LOOK)
                pv(i)

            rcp = fin.tile([128, TQ], F32, tag="rcp", name=f"rcp{h}")
            nc.vector.reciprocal_approx_fast(rcp[:], acc_d[:])
            nc.vector.tensor_mul(yt_sb[h][:], acc_y[:], rcp[:])

        # ---- phase C: O^T projection in e-tile pairs; wo streamed as
        # 8KB-row pair tiles (prefetch depth = wop bufs via queue ordering)
        for n0 in range(0, NCC, 2):
            np_ = n0 // 2
            wot = wop.tile([128, 4096], BF16, tag="wo", name=f"wot{np_}")
            nc.sync.dma_start(wot[:], wo[np_ * 128:(np_ + 1) * 128, :])
            opair = [ps_acc.tile([128, 512], F32, tag="acc",
                                 name=f"ops{n0}_{j}") for j in range(2)]
            for h in range(H):
                for j in range(2):
                    nc.tensor.matmul(opair[j][:],
                                     wot[:, h * 256 + j * 128:
                                         h * 256 + (j + 1) * 128],
                                     yt_sb[h][:],
                                     start=(h == 0), stop=(h == H - 1))
            for j in range(2):
                osb = fin.tile([128, TQ], F32, tag="osb", name=f"osb{n0}_{j}")
                nc.scalar.activation(osb[:], opair[j][:], Copy)
                nc.sync.dma_start(outT[(n0 + j) * 128:(n0 + j + 1) * 128, :],
                                  osb[:])


# ---------------------------------------------------------------- host side
def _host_inputs(x, Wq, Wk, Wv, Wo):
    x = np.asarray(x, dtype=np.float32).reshape(T, DIM)

    inv_freq = 1.0 / (ROPE_BASE ** (np.arange(0, D, 2, dtype=np.float64) / D))
    dfreq = np.concatenate([inv_freq, inv_freq])  # [128] per-dim freq

    # fat-row packed weight layouts (see _build comments)
    wq_r = np.ascontiguousarray(
        np.asarray(Wq).reshape(NCC, 128, 8, 256).transpose(2, 1, 0, 3)
        .reshape(8 * 128, 4096)).astype(BF)
    wo_r = np.ascontiguousarray(
        np.asarray(Wo).reshape(H, 128, 8, 256).transpose(2, 1, 0, 3)
        .reshape(8 * 128, 4096)).astype(BF)
    wk_r = np.ascontiguousarray(
        np.asarray(Wk, np.float32).reshape(NCC, 128, 512).transpose(1, 0, 2)
        .reshape(128, NCC * 512)).astype(BF)
    wv_r = np.ascontiguousarray(
        np.asarray(Wv, np.float32).reshape(NCC, 128, 512).transpose(1, 0, 2)
        .reshape(128, NCC * 512)).astype(BF)

    u = np.arange(128)[:, None]
    maskB = (np.arange(128)[None, :] < u).astype(BF)        # qq>=u -> 0
    maskA = (u <= np.arange(128)[None, :]).astype(BF)       # u>qq -> 0

    rotp = np.zeros((128, 128), np.float32)
    d = np.arange(128)
    rotp[(d + 64) % 128, d] = 1.0  # out[d] = in[(d+64)%128]
    rotp = rotp.astype(BF)

    ones = np.ones((128, 128), BF)

    in_maps = []
    for c in range(N_CORES):
        qs = c * TQ
        xqc = x[qs:qs + TQ]                     # [512, 2048]
        xkv = np.zeros((TKV, DIM), np.float32)  # [1536, 2048]
        lo = qs - WIN
        src_lo = max(0, lo)
        xkv[src_lo - lo:TKV] = x[src_lo:qs + TQ]

        pos_q = np.arange(qs, qs + TQ, dtype=np.float64)
        pos_k = np.arange(lo, qs + TQ, dtype=np.float64)
        angq = dfreq[:, None] * pos_q[None, :]  # [128, 512]
        angk = dfreq[:, None] * pos_k[None, :]  # [128, 1536]
        sgn = np.where(np.arange(D) < D // 2, -1.0, 1.0)[:, None]

        kb = np.zeros((128, NMT), np.float32)
        for m in range(NMT):
            t_abs = 128 * m + np.arange(128)
            kb[:, m] = np.where(t_abs < WIN - qs, -30.0, 0.0)

        in_maps.append({
            "xq": np.ascontiguousarray(
                xqc.T.reshape(NCC, 128, TQ).transpose(1, 0, 2)
                .reshape(128, NCC * TQ)).astype(BF),
            "xkvT": np.ascontiguousarray(
                xkv.T.reshape(DIM, 3, 512).transpose(1, 0, 2)
                .reshape(3 * DIM, 512)).astype(BF),
            "wq": wq_r, "wk": wk_r, "wv": wv_r, "wo": wo_r,
            "cosq": np.cos(angq).astype(np.float32),
            "sinq": (sgn * np.sin(angq)).astype(np.float32),
            "cosk": np.ascontiguousarray(np.cos(angk).astype(np.float32)
                .reshape(D, 3, 512).transpose(1, 0, 2)).reshape(3 * D, 512),
            "sink": np.ascontiguousarray(((sgn * np.sin(angk)).astype(np.float32))
                .reshape(D, 3, 512).transpose(1, 0, 2)).reshape(3 * D, 512),
            "kbias": kb,
            "maskB": maskB, "maskA": maskA,
            "rotp": rotp,
            "ones": ones,
        })
    return in_maps


def kernel(x, Wq, Wk, Wv, Wo, _trace=False, _trace_kwargs=None):
    nc = _build()
    in_maps = _host_inputs(x, Wq, Wk, Wv, Wo)
    res = run_bass_kernel_spmd(nc, in_maps, core_ids=list(range(N_CORES)),
                               trace=_trace, **(_trace_kwargs or {}))
    out = np.empty((1, T, DIM), np.float32)
    for c in range(N_CORES):
        out[0, c * TQ:(c + 1) * TQ, :] = res.results[c]["outT"].T
    if _trace:
        kernel.last_results = res
    return out


# revision 17
# speedup vs baseline: 1.5813x; 1.0371x over previous
"""Sliding-window GQA attention (T=4096, DIM=2048, H=16, KVH=4, D=128, W=1024)
as an 8-core SPMD Trainium2 Bass/Tile kernel.

Sharding: sequence-parallel. Core c owns queries [512c, 512c+512) and
recomputes K/V for its sliding window (1536 kv slots, zero-padded before
position 0). No collectives.

v3: all matmul operands bf16 (PSUM fp32), exact attention spans, four
phases, fat-row DMA layouts (weights packed so each DMA moves 8-16KB per
partition line instead of 512B descriptors), software-pipelined RoPE
(rope matmuls of pair p emitted mid-chain of pair p+1), deferred g=3
K-chain start to hide cross-span PSUM WAR waits:
  A : K^T (RoPE'd, bf16) and V (natural, bf16) over 3 spans of 512 kv slots
  A2: all 16 Q^T heads projected + RoPE'd
  B : attention, S(m+2) issued before PV(m)/den(m); LOOK=2
  C : O^T projection, wo streamed as 8KB-row pair tiles
Softmax denominator reciprocal via fast custom-DVE op; PSUM->SBUF copies
on the scalar (ACT) engine.
"""

import math
import os
import sys

import numpy as np


def _ensure_paths():
    for p in (
        "/root/.axon_site",
        "/root/.axon_site/_ro/trn_rl_repo",
        "/root/.axon_site/_ro/pypackages",
        "/opt/trn_rl_repo",
        "/opt/pypackages",
    ):
        if os.path.isdir(p) and p not in sys.path:
            sys.path.append(p)


try:
    import concourse.bass as bass  # noqa: F401
except ImportError:
    _ensure_paths()

import ml_dtypes

import concourse.bass as bass
import concourse.mybir as mybir
import concourse.tile as tile
from concourse import bacc
from concourse.bass_utils import run_bass_kernel_spmd

# ---------------------------------------------------------------- constants
N_CORES = 8
T = 4096
DIM = 2048
H = 16
KVH = 4
D = 128
WIN = 1024
ROPE_BASE = 10000.0

TQ = T // N_CORES          # 512 queries per core
TKV = TQ + WIN             # 1536 kv slots per core
NMT = TKV // 128           # 12 kv tiles of 128
NCC = DIM // 128           # 16 contraction chunks
SCALE = 1.0 / math.sqrt(D)
GQ = H // KVH              # 4 q heads per kv head

F32 = mybir.dt.float32
BF16 = mybir.dt.bfloat16
BF = ml_dtypes.bfloat16

# per kv-tile m: exact (qlo, qhi) span of local queries it can interact with
SPANS = {
    0: (0, 128), 1: (0, 256), 2: (0, 384), 3: (0, 512),
    4: (0, 512), 5: (0, 512), 6: (0, 512), 7: (0, 512),
    8: (0, 512), 9: (128, 512), 10: (256, 512), 11: (384, 512),
}
# per kv-tile m: (mask_name, lo, hi) triangle block in absolute q coords
MASKS = {
    0: ("maskB", 0, 128), 1: ("maskB", 128, 256),
    2: ("maskB", 256, 384), 3: ("maskB", 384, 512),
    4: None, 5: None, 6: None, 7: None,
    8: ("maskA", 0, 128), 9: ("maskA", 128, 256),
    10: ("maskA", 256, 384), 11: ("maskA", 384, 512),
}
# Phase-B pipeline units: each is one PSUM S-bank + one exp. Narrow tiles
# are merged pairwise (their S spans packed side by side in one bank) to
# amortize ACT per-instruction overhead. Padding on merged tiles is handled
# by the valid-vector denominator (K=0 -> P=exp(0)=1, V=0, valid=0) instead
# of kbias. Unit {4} first (full-width start=True), {2,11} last (stop on 11).
UNITS = [(4,), (5,), (6,), (7,), (0, 1), (3,), (8,), (9,), (10,), (2, 11)]
UOFF = {0: 0, 1: 128, 2: 0, 11: 384}   # col offset of m inside its unit bank
MERGED = {0: 0, 1: 1, 2: 2, 11: 3}     # m -> index into the valid table
LOOK = 3                   # S-unit lookahead depth in phase B


# ---------------------------------------------------------------- device code
_NC_CACHE = None


def _build():
    global _NC_CACHE
    if _NC_CACHE is not None:
        return _NC_CACHE

    nc = bacc.Bacc("TRN2", target_bir_lowering=False, debug=False,
                   num_devices=N_CORES)

    # DRAM I/O (per-core contents supplied via in_maps). Weight layouts are
    # packed so every DMA moves a fat contiguous row per partition:
    #   wq[p*128+r, c*256+jc]   = Wq[c*128+r, p*256+jc]     (8KB rows)
    #   wo[np*128+r, h*256+jc]  = Wo[h*128+r, np*256+jc]    (8KB rows)
    #   wk[r, c*512+e]          = Wk[c*128+r, e]            (16KB rows)
    #   wv[r, c*512+e]          = Wv[c*128+r, e]            (16KB rows)
    #   xq[r, c*512+q]          = x[qs+q, c*128+r]          (16KB rows)
    xq = nc.dram_tensor("xq", [128, NCC * TQ], BF16, kind="ExternalInput").ap()
    xkvT = nc.dram_tensor("xkvT", [3 * DIM, 512], BF16, kind="ExternalInput").ap()
    wq = nc.dram_tensor("wq", [8 * 128, 4096], BF16, kind="ExternalInput").ap()
    wk = nc.dram_tensor("wk", [128, NCC * 512], BF16, kind="ExternalInput").ap()
    wv = nc.dram_tensor("wv", [128, NCC * 512], BF16, kind="ExternalInput").ap()
    wo = nc.dram_tensor("wo", [8 * 128, 4096], BF16, kind="ExternalInput").ap()
    cosq = nc.dram_tensor("cosq", [D, TQ], F32, kind="ExternalInput").ap()
    sinq = nc.dram_tensor("sinq", [D, TQ], F32, kind="ExternalInput").ap()
    cosk = nc.dram_tensor("cosk", [3 * D, 512], F32, kind="ExternalInput").ap()
    sink = nc.dram_tensor("sink", [3 * D, 512], F32, kind="ExternalInput").ap()
    kbias = nc.dram_tensor("kbias", [128, NMT], F32, kind="ExternalInput").ap()
    valid = nc.dram_tensor("valid", [128, 512], BF16, kind="ExternalInput").ap()
    maskB = nc.dram_tensor("maskB", [128, 128], BF16, kind="ExternalInput").ap()
    maskA = nc.dram_tensor("maskA", [128, 128], BF16, kind="ExternalInput").ap()
    rotp = nc.dram_tensor("rotp", [128, 128], BF16, kind="ExternalInput").ap()
    ones = nc.dram_tensor("ones", [128, 128], BF16, kind="ExternalInput").ap()
    outT = nc.dram_tensor("outT", [DIM, TQ], F32, kind="ExternalOutput").ap()

    mask_dram = {"maskB": maskB, "maskA": maskA}

    with tile.TileContext(nc) as tc:
        _emit(nc, tc, xq, xkvT, wq, wk, wv, wo, cosq, sinq, cosk, sink,
              kbias, valid, mask_dram, rotp, ones, outT)

    nc.compile()
    _NC_CACHE = nc
    return nc


def _emit(nc, tc, xq, xkvT, wq, wk, wv, wo, cosq, sinq, cosk, sink,
          kbias, valid, mask_dram, rotp, ones, outT):
    from contextlib import ExitStack

    ctx = ExitStack()
    with ctx:
        # SBUF pools (sizes are per-partition bytes; total ~202KB < 208KB)
        consts = ctx.enter_context(tc.tile_pool(name="consts", bufs=1))
        ropet = ctx.enter_context(tc.tile_pool(name="ropet", bufs=4))
        xsp = ctx.enter_context(tc.tile_pool(name="xsp", bufs=17))
        wkp = ctx.enter_context(tc.tile_pool(name="wkp", bufs=1))
        wvp = ctx.enter_context(tc.tile_pool(name="wvp", bufs=1))
        wqp = ctx.enter_context(tc.tile_pool(name="wqp", bufs=3))
        wop = ctx.enter_context(tc.tile_pool(name="wop", bufs=3))
        xqp = ctx.enter_context(tc.tile_pool(name="xqp", bufs=1))
        qtp = ctx.enter_context(tc.tile_pool(name="qtp", bufs=H))
        ktp = ctx.enter_context(tc.tile_pool(name="ktp", bufs=KVH))
        vp = ctx.enter_context(tc.tile_pool(name="vp", bufs=NMT))
        ytp = ctx.enter_context(tc.tile_pool(name="ytp", bufs=H))
        pp = ctx.enter_context(tc.tile_pool(name="pp", bufs=5))
        tmp = ctx.enter_context(tc.tile_pool(name="tmp", bufs=4))
        t12 = ctx.enter_context(tc.tile_pool(name="t12", bufs=4))
        fin = ctx.enter_context(tc.tile_pool(name="fin", bufs=2))
        # PSUM: exactly 8 banks
        ps_acc = ctx.enter_context(tc.tile_pool(name="ps_acc", bufs=4, space="PSUM"))
        ps_s = ctx.enter_context(tc.tile_pool(name="ps_s", bufs=3, space="PSUM"))
        ps_r = ctx.enter_context(tc.tile_pool(name="ps_r", bufs=1, space="PSUM"))

        Exp = mybir.ActivationFunctionType.Exp
        Copy = mybir.ActivationFunctionType.Copy

        # ---- persistent weights (gpsimd DMA queue). Split into 4 sub-tiles
        # each so the first K/V chains start after ~512KB instead of 2MB,
        # and early xs tiles on the sync queue are not starved.
        # wk split [2,2,4,4,4] c-chunks (smaller first pieces -> earlier
        # first matmul), wv split 4x4
        wk_sub = {}      # c -> (tile, col_base)
        wk_splits = [(0, 2), (2, 2), (4, 4), (8, 4), (12, 4)]
        for si, (c0, ncs) in enumerate(wk_splits):
            wkt = wkp.tile([128, ncs * 512], BF16, tag=f"wk{si}",
                           name=f"wk_sub{si}", bufs=1)
            nc.gpsimd.dma_start(wkt[:], wk[:, c0 * 512:(c0 + ncs) * 512])
            for c in range(c0, c0 + ncs):
                wk_sub[c] = (wkt, (c - c0) * 512)
        wv_sub = []
        for q4 in range(4):
            wvt = wvp.tile([128, 4 * 512], BF16, tag="wv", name=f"wv_sub{q4}",
                           bufs=4)
            nc.gpsimd.dma_start(wvt[:], wv[:, q4 * 2048:(q4 + 1) * 2048])
            wv_sub.append(wvt)

        def wk_sl(c, g):
            t, base = wk_sub[c]
            return t[:, base + g * 128:base + (g + 1) * 128]

        def wv_sl(c):
            return wv_sub[c // 4][:, (c % 4) * 512:(c % 4 + 1) * 512]

        # ---- phase A: K^T (RoPE'd) and V over 3 spans of 512 kv slots
        kt_sb = [ktp.tile([128, TKV], BF16, tag="kt", name=f"kt{g}")
                 for g in range(KVH)]
        v_sb = [vp.tile([128, 512], BF16, tag="v", name=f"v{m}")
                for m in range(NMT)]

        consts_loaded = [False]
        const_sb = {}

        def cload(ap, shape, dtype, tag):
            t = consts.tile(shape, dtype, tag=tag, name=tag)
            nc.sync.dma_start(t[:], ap[:])
            return t

        for s in range(3):
            xs = []
            for c in range(NCC):
                xt = xsp.tile([128, 512], BF16, tag="xs", name=f"xs{s}_{c}")
                nc.sync.dma_start(
                    xt[:], xkvT[s * DIM + c * 128:s * DIM + (c + 1) * 128, :])
                xs.append(xt)
            cosk_s = ropet.tile([128, 512], F32, tag="rt", name=f"cosk{s}")
            nc.sync.dma_start(cosk_s[:], cosk[s * 128:(s + 1) * 128, :])
            sink_s = ropet.tile([128, 512], F32, tag="rt", name=f"sink{s}")
            nc.sync.dma_start(sink_s[:], sink[s * 128:(s + 1) * 128, :])
            if not consts_loaded[0]:
                const_sb["rotp"] = cload(rotp, [128, 128], BF16, "rotp")
                const_sb["ones"] = cload(ones, [128, 128], BF16, "ones")
                const_sb["kbias"] = cload(kbias, [128, NMT], F32, "kbias")
                const_sb["valid"] = cload(valid, [128, 512], BF16, "valid")
                const_sb["maskB"] = cload(mask_dram["maskB"], [128, 128],
                                          BF16, "maskB")
                const_sb["maskA"] = cload(mask_dram["maskA"], [128, 128],
                                          BF16, "maskA")
                consts_loaded[0] = True

            # K^T projection: 4 chains (one per kv head) across acc banks.
            # g=3's first writes are deferred 12 matmuls so the WAR on last
            # span's kps[3] (read late by its rope t2-mul) is hidden.
            kps = [ps_acc.tile([128, 512], F32, tag="acc", name=f"kps{s}_{g}")
                   for g in range(KVH)]

            def kmm(c, g):
                nc.tensor.matmul(kps[g][:], wk_sl(c, g), xs[c][:],
                                 start=(c == 0), stop=(c == NCC - 1))

            for c in range(4):
                for g in range(3):
                    kmm(c, g)
            for c in range(4):
                kmm(c, 3)
            for c in range(4, NCC):
                for g in range(KVH):
                    kmm(c, g)

            # rope rotate-half sources, copied early on ACT
            ssb = []
            for g in range(KVH):
                sg = tmp.tile([128, 512], BF16, tag="ssb", name=f"ssb{s}_{g}")
                nc.scalar.activation(sg[:], kps[g][:], Copy)
                ssb.append(sg)

            def ropek(g):
                r_ps = ps_r.tile([128, 512], F32, tag="rp", name=f"rk{s}_{g}")
                nc.tensor.matmul(r_ps[:], const_sb["rotp"][:], ssb[g][:],
                                 start=True, stop=True)
                t1 = t12.tile([128, 512], F32, tag="t12", name=f"kt1_{s}_{g}")
                nc.vector.tensor_mul(t1[:], r_ps[:], sink_s[:])
                t2 = t12.tile([128, 512], F32, tag="t12", name=f"kt2_{s}_{g}")
                nc.vector.tensor_mul(t2[:], kps[g][:], cosk_s[:])
                nc.vector.tensor_add(kt_sb[g][:, s * 512:(s + 1) * 512],
                                     t1[:], t2[:])

            # V projection (natural layout) in 2 passes of 2 PSUM banks,
            # with the 4 rope matmuls interleaved between V-chain batches
            vps = {}

            def vchain(tts, c0, c1):
                for c in range(c0, c1):
                    for tt in tts:
                        nc.tensor.matmul(
                            vps[tt][:],
                            xs[c][:, tt * 128:(tt + 1) * 128],
                            wv_sl(c),
                            start=(c == 0), stop=(c == NCC - 1))

            for tt in (0, 1):
                vps[tt] = ps_s.tile([128, 512], F32, tag="sps",
                                    name=f"vps{s}_{tt}")
            vchain((0, 1), 0, 8)
            ropek(0)
            vchain((0, 1), 8, NCC)
            ropek(1)
            for tt in (0, 1):
                nc.scalar.activation(v_sb[4 * s + tt][:], vps[tt][:], Copy)
            for tt in (2, 3):
                vps[tt] = ps_s.tile([128, 512], F32, tag="sps",
                                    name=f"vps{s}_{tt}")
            vchain((2, 3), 0, 4)
            ropek(2)
            vchain((2, 3), 4, 8)
            ropek(3)
            vchain((2, 3), 8, NCC)
            for tt in (2, 3):
                nc.scalar.activation(v_sb[4 * s + tt][:], vps[tt][:], Copy)

        # ---- phase A2: all 16 Q^T heads projected + RoPE'd.
        # Rope matmuls of pair p are emitted mid-chain of pair p+1 so the
        # PE never waits on the ACT rotate-source copy.
        xq_all = xqp.tile([128, NCC * 512], BF16, tag="xq", name="xq_all")
        nc.sync.dma_start(xq_all[:], xq[:])
        cosq_sb = ropet.tile([128, 512], F32, tag="rt", name="cosq")
        nc.sync.dma_start(cosq_sb[:], cosq[:])
        sinq_sb = ropet.tile([128, 512], F32, tag="rt", name="sinq")
        nc.sync.dma_start(sinq_sb[:], sinq[:])

        qts = {}

        def ropeq(p_, j, qpair):
            sg = tmp.tile([128, 512], BF16, tag="ssb", name=f"sq{p_}_{j}")
            nc.scalar.activation(sg[:], qpair[j][:], Copy)
            r_ps = ps_s.tile([128, 512], F32, tag="sps", name=f"rq{p_}_{j}")
            nc.tensor.matmul(r_ps[:], const_sb["rotp"][:], sg[:],
                             start=True, stop=True)
            t1 = t12.tile([128, 512], F32, tag="t12", name=f"qt1_{p_}_{j}")
            nc.vector.tensor_mul(t1[:], r_ps[:], sinq_sb[:])
            t2 = t12.tile([128, 512], F32, tag="t12", name=f"qt2_{p_}_{j}")
            nc.vector.tensor_mul(t2[:], qpair[j][:], cosq_sb[:])
            qtj = qtp.tile([128, 512], BF16, tag="qt", name=f"qt{2 * p_ + j}")
            nc.vector.tensor_add(qtj[:], t1[:], t2[:])
            qts[2 * p_ + j] = qtj

        prev = None  # (p_, qpair) whose ropes are pending
        for p_ in range(H // 2):
            qpair = [ps_acc.tile([128, 512], F32, tag="acc",
                                 name=f"qps{p_}_{j}") for j in range(2)]
            # sync queue: sits behind all xs tiles, so these 1MB transfers
            # cannot starve phase A's time-critical loads
            wqt = wqp.tile([128, 4096], BF16, tag="wq", name=f"wqt{p_}")
            nc.sync.dma_start(wqt[:], wq[p_ * 128:(p_ + 1) * 128, :])
            for c in range(NCC):
                if c == 6 and prev is not None:
                    ropeq(prev[0], 0, prev[1])
                if c == 10 and prev is not None:
                    ropeq(prev[0], 1, prev[1])
                    prev = None
                for j in range(2):
                    nc.tensor.matmul(qpair[j][:],
                                     wqt[:, c * 256 + j * 128:
                                         c * 256 + (j + 1) * 128],
                                     xq_all[:, c * 512:(c + 1) * 512],
                                     start=(c == 0), stop=(c == NCC - 1))
            prev = (p_, qpair)
        ropeq(prev[0], 0, prev[1])
        ropeq(prev[0], 1, prev[1])

        # ---- phase B: attention, software-pipelined per head
        yt_sb = [ytp.tile([128, TQ], BF16, tag="yt", name=f"yt{h}")
                 for h in range(H)]

        ucount = [0]
        for h in range(H):
            g = h // GQ
            qt = qts[h]
            acc_y = ps_acc.tile([128, TQ], F32, tag="acc", name=f"yps{h}")
            acc_d = ps_acc.tile([128, TQ], F32, tag="acc", name=f"dps{h}")
            p_l = {}

            def qk(ui, h=h, g=g, qt=qt, p_l=p_l):
                unit = UNITS[ui]
                merged = len(unit) > 1
                # rotate S banks over ps_s (3) + the A-phase rope bank (1)
                u = ucount[0]
                ucount[0] += 1
                pool = ps_r if u % 4 == 3 else ps_s
                tagn = "rp" if u % 4 == 3 else "sps"
                sps = pool.tile([128, 512], F32, tag=tagn,
                                name=f"sps{h}_{unit[0]}")
                ext = 0
                for m in unit:
                    qlo, qhi = SPANS[m]
                    w = qhi - qlo
                    off = UOFF[m] if merged else 0
                    nc.tensor.matmul(sps[:, off:off + w],
                                     kt_sb[g][:, m * 128:(m + 1) * 128],
                                     qt[:, qlo:qhi], start=True, stop=True)
                    ext = max(ext, off + w)
                p = pp.tile([128, 512], BF16, tag="p", name=f"p{h}_{unit[0]}")
                bias = 0.0 if merged else const_sb["kbias"][:, unit[0]:
                                                            unit[0] + 1]
                nc.scalar.activation(p[:, :ext], sps[:, :ext], Exp,
                                     bias=bias, scale=SCALE)
                for m in unit:
                    mk = MASKS[m]
                    if mk is not None:
                        qlo, qhi = SPANS[m]
                        off = (UOFF[m] if merged else 0) - qlo
                        name_, lo, hi = mk
                        nc.vector.tensor_mul(p[:, lo + off:hi + off],
                                             p[:, lo + off:hi + off],
                                             const_sb[name_][:])
                p_l[ui] = p

            def pv(ui, h=h, g=g, acc_y=acc_y, acc_d=acc_d, p_l=p_l):
                unit = UNITS[ui]
                merged = len(unit) > 1
                p = p_l.pop(ui)
                first = ui == 0
                last_unit = ui == len(UNITS) - 1
                for m in unit:
                    qlo, qhi = SPANS[m]
                    w = qhi - qlo
                    off = UOFF[m] if merged else 0
                    last = last_unit and m == unit[-1]
                    if m in MERGED:
                        vi = MERGED[m]
                        den_st = const_sb["valid"][:, vi * 128:(vi + 1) * 128]
                    else:
                        den_st = const_sb["ones"][:]
                    nc.tensor.matmul(acc_y[:, qlo:qhi],
                                     v_sb[m][:, g * 128:(g + 1) * 128],
                                     p[:, off:off + w], start=first,
                                     stop=last)
                    nc.tensor.matmul(acc_d[:, qlo:qhi], den_st,
                                     p[:, off:off + w], start=first,
                                     stop=last)
                    first = False

            for i in range(LOOK):
                qk(i)
            for i in range(len(UNITS)):
                if i + LOOK < len(UNITS):
                    qk(i + LOOK)
                pv(i)

            rcp = fin.tile([128, TQ], F32, tag="rcp", name=f"rcp{h}")
            nc.vector.reciprocal_approx_fast(rcp[:], acc_d[:])
            nc.vector.tensor_mul(yt_sb[h][:], acc_y[:], rcp[:])

        # ---- phase C: O^T projection in e-tile pairs; wo streamed as
        # 8KB-row pair tiles (prefetch depth = wop bufs via queue ordering)
        for n0 in range(0, NCC, 2):
            np_ = n0 // 2
            wot = wop.tile([128, 4096], BF16, tag="wo", name=f"wot{np_}")
            nc.sync.dma_start(wot[:], wo[np_ * 128:(np_ + 1) * 128, :])
            opair = [ps_acc.tile([128, 512], F32, tag="acc",
                                 name=f"ops{n0}_{j}") for j in range(2)]
            for h in range(H):
                for j in range(2):
                    nc.tensor.matmul(opair[j][:],
                                     wot[:, h * 256 + j * 128:
                                         h * 256 + (j + 1) * 128],
                                     yt_sb[h][:],
                                     start=(h == 0), stop=(h == H - 1))
            for j in range(2):
                osb = fin.tile([128, TQ], F32, tag="osb", name=f"osb{n0}_{j}")
                nc.scalar.activation(osb[:], opair[j][:], Copy)
                nc.sync.dma_start(outT[(n0 + j) * 128:(n0 + j + 1) * 128, :],
                                  osb[:])


# ---------------------------------------------------------------- host side
def _host_inputs(x, Wq, Wk, Wv, Wo):
    x = np.asarray(x, dtype=np.float32).reshape(T, DIM)

    inv_freq = 1.0 / (ROPE_BASE ** (np.arange(0, D, 2, dtype=np.float64) / D))
    dfreq = np.concatenate([inv_freq, inv_freq])  # [128] per-dim freq

    # fat-row packed weight layouts (see _build comments)
    wq_r = np.ascontiguousarray(
        np.asarray(Wq).reshape(NCC, 128, 8, 256).transpose(2, 1, 0, 3)
        .reshape(8 * 128, 4096)).astype(BF)
    wo_r = np.ascontiguousarray(
        np.asarray(Wo).reshape(H, 128, 8, 256).transpose(2, 1, 0, 3)
        .reshape(8 * 128, 4096)).astype(BF)
    wk_r = np.ascontiguousarray(
        np.asarray(Wk, np.float32).reshape(NCC, 128, 512).transpose(1, 0, 2)
        .reshape(128, NCC * 512)).astype(BF)
    wv_r = np.ascontiguousarray(
        np.asarray(Wv, np.float32).reshape(NCC, 128, 512).transpose(1, 0, 2)
        .reshape(128, NCC * 512)).astype(BF)

    u = np.arange(128)[:, None]
    maskB = (np.arange(128)[None, :] < u).astype(BF)        # qq>=u -> 0
    maskA = (u <= np.arange(128)[None, :]).astype(BF)       # u>qq -> 0

    rotp = np.zeros((128, 128), np.float32)
    d = np.arange(128)
    rotp[(d + 64) % 128, d] = 1.0  # out[d] = in[(d+64)%128]
    rotp = rotp.astype(BF)

    ones = np.ones((128, 128), BF)

    in_maps = []
    for c in range(N_CORES):
        qs = c * TQ
        xqc = x[qs:qs + TQ]                     # [512, 2048]
        xkv = np.zeros((TKV, DIM), np.float32)  # [1536, 2048]
        lo = qs - WIN
        src_lo = max(0, lo)
        xkv[src_lo - lo:TKV] = x[src_lo:qs + TQ]

        pos_q = np.arange(qs, qs + TQ, dtype=np.float64)
        pos_k = np.arange(lo, qs + TQ, dtype=np.float64)
        angq = dfreq[:, None] * pos_q[None, :]  # [128, 512]
        angk = dfreq[:, None] * pos_k[None, :]  # [128, 1536]
        sgn = np.where(np.arange(D) < D // 2, -1.0, 1.0)[:, None]

        kb = np.zeros((128, NMT), np.float32)
        for m in range(NMT):
            t_abs = 128 * m + np.arange(128)
            kb[:, m] = np.where(t_abs < WIN - qs, -30.0, 0.0)
        # valid-vector den stationaries for the merged exp tiles {0,1,2,11}:
        # stat[t, i] = 1 unless kv slot 128m+t is left-padding on this core
        vld = np.zeros((128, 512), np.float32)
        for vi, m in enumerate((0, 1, 2, 11)):
            t_abs = 128 * m + np.arange(128)
            vld[:, vi * 128:(vi + 1) * 128] = np.where(
                t_abs < WIN - qs, 0.0, 1.0)[:, None]

        in_maps.append({
            "xq": np.ascontiguousarray(
                xqc.T.reshape(NCC, 128, TQ).transpose(1, 0, 2)
                .reshape(128, NCC * TQ)).astype(BF),
            "xkvT": np.ascontiguousarray(
                xkv.T.reshape(DIM, 3, 512).transpose(1, 0, 2)
                .reshape(3 * DIM, 512)).astype(BF),
            "wq": wq_r, "wk": wk_r, "wv": wv_r, "wo": wo_r,
            "cosq": np.cos(angq).astype(np.float32),
            "sinq": (sgn * np.sin(angq)).astype(np.float32),
            "cosk": np.ascontiguousarray(np.cos(angk).astype(np.float32)
                .reshape(D, 3, 512).transpose(1, 0, 2)).reshape(3 * D, 512),
            "sink": np.ascontiguousarray(((sgn * np.sin(angk)).astype(np.float32))
                .reshape(D, 3, 512).transpose(1, 0, 2)).reshape(3 * D, 512),
            "kbias": kb,
            "valid": vld.astype(BF),
            "maskB": maskB, "maskA": maskA,
            "rotp": rotp,
            "ones": ones,
        })
    return in_maps


def kernel(x, Wq, Wk, Wv, Wo, _trace=False, _trace_kwargs=None):
    nc = _build()
    in_maps = _host_inputs(x, Wq, Wk, Wv, Wo)
    res = run_bass_kernel_spmd(nc, in_maps, core_ids=list(range(N_CORES)),
                               trace=_trace, **(_trace_kwargs or {}))
    out = np.empty((1, T, DIM), np.float32)
    for c in range(N_CORES):
        out[0, c * TQ:(c + 1) * TQ, :] = res.results[c]["outT"].T
    if _trace:
        kernel.last_results = res
    return out


# revision 21
# speedup vs baseline: 1.5986x; 1.0109x over previous
"""Sliding-window GQA attention (T=4096, DIM=2048, H=16, KVH=4, D=128, W=1024)
as an 8-core SPMD Trainium2 Bass/Tile kernel.

Sharding: sequence-parallel. Core c owns queries [512c, 512c+512) and
recomputes K/V for its sliding window (1536 kv slots, zero-padded before
position 0). No collectives.

v3: all matmul operands bf16 (PSUM fp32), exact attention spans, four
phases, fat-row DMA layouts (weights packed so each DMA moves 8-16KB per
partition line instead of 512B descriptors), software-pipelined RoPE
(rope matmuls of pair p emitted mid-chain of pair p+1), deferred g=3
K-chain start to hide cross-span PSUM WAR waits:
  A : K^T (RoPE'd, bf16) and V (natural, bf16) over 3 spans of 512 kv slots
  A2: all 16 Q^T heads projected + RoPE'd
  B : attention, S(m+2) issued before PV(m)/den(m); LOOK=2
  C : O^T projection, wo streamed as 8KB-row pair tiles
Softmax denominator reciprocal via fast custom-DVE op; PSUM->SBUF copies
on the scalar (ACT) engine.
"""

import math
import os
import sys

import numpy as np


def _ensure_paths():
    for p in (
        "/root/.axon_site",
        "/root/.axon_site/_ro/trn_rl_repo",
        "/root/.axon_site/_ro/pypackages",
        "/opt/trn_rl_repo",
        "/opt/pypackages",
    ):
        if os.path.isdir(p) and p not in sys.path:
            sys.path.append(p)


try:
    import concourse.bass as bass  # noqa: F401
except ImportError:
    _ensure_paths()

import ml_dtypes

import concourse.bass as bass
import concourse.mybir as mybir
import concourse.tile as tile
from concourse import bacc
from concourse.bass_utils import run_bass_kernel_spmd

# ---------------------------------------------------------------- constants
N_CORES = 8
T = 4096
DIM = 2048
H = 16
KVH = 4
D = 128
WIN = 1024
ROPE_BASE = 10000.0

TQ = T // N_CORES          # 512 queries per core
TKV = TQ + WIN             # 1536 kv slots per core
NMT = TKV // 128           # 12 kv tiles of 128
NCC = DIM // 128           # 16 contraction chunks
SCALE = 1.0 / math.sqrt(D)
GQ = H // KVH              # 4 q heads per kv head

F32 = mybir.dt.float32
BF16 = mybir.dt.bfloat16
BF = ml_dtypes.bfloat16

# per kv-tile m: exact (qlo, qhi) span of local queries it can interact with
SPANS = {
    0: (0, 128), 1: (0, 256), 2: (0, 384), 3: (0, 512),
    4: (0, 512), 5: (0, 512), 6: (0, 512), 7: (0, 512),
    8: (0, 512), 9: (128, 512), 10: (256, 512), 11: (384, 512),
}
# per kv-tile m: (mask_name, lo, hi) triangle block in absolute q coords
MASKS = {
    0: ("maskB", 0, 128), 1: ("maskB", 128, 256),
    2: ("maskB", 256, 384), 3: ("maskB", 384, 512),
    4: None, 5: None, 6: None, 7: None,
    8: ("maskA", 0, 128), 9: ("maskA", 128, 256),
    10: ("maskA", 256, 384), 11: ("maskA", 384, 512),
}
# Phase-B pipeline units: each is one PSUM S-bank + one exp. Narrow tiles
# are merged pairwise (their S spans packed side by side in one bank) to
# amortize ACT per-instruction overhead. Padding on merged tiles is handled
# by the valid-vector denominator (K=0 -> P=exp(0)=1, V=0, valid=0) instead
# of kbias. Unit {4} first (full-width start=True), {2,11} last (stop on 11).
UNITS = [(4,), (5,), (6,), (7,), (9, 0), (10, 1), (3,), (8,), (2, 11)]
UOFF = {9: 0, 0: 384, 10: 0, 1: 256, 2: 0, 11: 384}  # col offset in unit bank
# m -> valid-table block (m9/m10/m11 are never left-padding: plain ones)
MERGED = {0: 0, 1: 1, 2: 2}
LOOK = 3                   # S-unit lookahead depth in phase B


# ---------------------------------------------------------------- device code
_NC_CACHE = None


def _build():
    global _NC_CACHE
    if _NC_CACHE is not None:
        return _NC_CACHE

    nc = bacc.Bacc("TRN2", target_bir_lowering=False, debug=False,
                   num_devices=N_CORES)

    # DRAM I/O (per-core contents supplied via in_maps). Weight layouts are
    # packed so every DMA moves a fat contiguous row per partition:
    #   wq[p*128+r, c*256+jc]   = Wq[c*128+r, p*256+jc]     (8KB rows)
    #   wo[np*128+r, h*256+jc]  = Wo[h*128+r, np*256+jc]    (8KB rows)
    #   wk[r, c*512+e]          = Wk[c*128+r, e]            (16KB rows)
    #   wv[r, c*512+e]          = Wv[c*128+r, e]            (16KB rows)
    #   xq[r, c*512+q]          = x[qs+q, c*128+r]          (16KB rows)
    xq = nc.dram_tensor("xq", [128, NCC * TQ], BF16, kind="ExternalInput").ap()
    xkvT = nc.dram_tensor("xkvT", [3 * DIM, 512], BF16, kind="ExternalInput").ap()
    wq = nc.dram_tensor("wq", [8 * 128, 4096], BF16, kind="ExternalInput").ap()
    wk = nc.dram_tensor("wk", [128, NCC * 512], BF16, kind="ExternalInput").ap()
    wv = nc.dram_tensor("wv", [128, NCC * 512], BF16, kind="ExternalInput").ap()
    wo = nc.dram_tensor("wo", [8 * 128, 4096], BF16, kind="ExternalInput").ap()
    cosq = nc.dram_tensor("cosq", [D, TQ], F32, kind="ExternalInput").ap()
    sinq = nc.dram_tensor("sinq", [D, TQ], F32, kind="ExternalInput").ap()
    cosk = nc.dram_tensor("cosk", [3 * D, 512], F32, kind="ExternalInput").ap()
    sink = nc.dram_tensor("sink", [3 * D, 512], F32, kind="ExternalInput").ap()
    kbias = nc.dram_tensor("kbias", [128, NMT], F32, kind="ExternalInput").ap()
    valid = nc.dram_tensor("valid", [128, 384], BF16, kind="ExternalInput").ap()
    maskB = nc.dram_tensor("maskB", [128, 128], BF16, kind="ExternalInput").ap()
    maskA = nc.dram_tensor("maskA", [128, 128], BF16, kind="ExternalInput").ap()
    rotp = nc.dram_tensor("rotp", [128, 128], BF16, kind="ExternalInput").ap()
    ones = nc.dram_tensor("ones", [128, 128], BF16, kind="ExternalInput").ap()
    outT = nc.dram_tensor("outT", [DIM, TQ], F32, kind="ExternalOutput").ap()

    mask_dram = {"maskB": maskB, "maskA": maskA}

    with tile.TileContext(nc) as tc:
        _emit(nc, tc, xq, xkvT, wq, wk, wv, wo, cosq, sinq, cosk, sink,
              kbias, valid, mask_dram, rotp, ones, outT)

    nc.compile()
    _NC_CACHE = nc
    return nc


def _emit(nc, tc, xq, xkvT, wq, wk, wv, wo, cosq, sinq, cosk, sink,
          kbias, valid, mask_dram, rotp, ones, outT):
    from contextlib import ExitStack

    ctx = ExitStack()
    with ctx:
        # SBUF pools (sizes are per-partition bytes; total ~202KB < 208KB)
        consts = ctx.enter_context(tc.tile_pool(name="consts", bufs=1))
        ropet = ctx.enter_context(tc.tile_pool(name="ropet", bufs=4))
        xsp = ctx.enter_context(tc.tile_pool(name="xsp", bufs=17))
        wkp = ctx.enter_context(tc.tile_pool(name="wkp", bufs=1))
        wvp = ctx.enter_context(tc.tile_pool(name="wvp", bufs=1))
        wqp = ctx.enter_context(tc.tile_pool(name="wqp", bufs=3))
        wop = ctx.enter_context(tc.tile_pool(name="wop", bufs=3))
        xqp = ctx.enter_context(tc.tile_pool(name="xqp", bufs=1))
        qtp = ctx.enter_context(tc.tile_pool(name="qtp", bufs=H))
        ktp = ctx.enter_context(tc.tile_pool(name="ktp", bufs=KVH))
        vp = ctx.enter_context(tc.tile_pool(name="vp", bufs=NMT))
        ytp = ctx.enter_context(tc.tile_pool(name="ytp", bufs=H))
        pp = ctx.enter_context(tc.tile_pool(name="pp", bufs=5))
        tmp = ctx.enter_context(tc.tile_pool(name="tmp", bufs=4))
        t12 = ctx.enter_context(tc.tile_pool(name="t12", bufs=4))
        fin = ctx.enter_context(tc.tile_pool(name="fin", bufs=2))
        # PSUM: exactly 8 banks
        ps_acc = ctx.enter_context(tc.tile_pool(name="ps_acc", bufs=4, space="PSUM"))
        ps_s = ctx.enter_context(tc.tile_pool(name="ps_s", bufs=3, space="PSUM"))
        ps_r = ctx.enter_context(tc.tile_pool(name="ps_r", bufs=1, space="PSUM"))

        Exp = mybir.ActivationFunctionType.Exp
        Copy = mybir.ActivationFunctionType.Copy

        # ---- persistent weights (gpsimd DMA queue). Split into 4 sub-tiles
        # each so the first K/V chains start after ~512KB instead of 2MB,
        # and early xs tiles on the sync queue are not starved.
        # wk split [2,2,4,4,4] c-chunks (smaller first pieces -> earlier
        # first matmul), wv split 4x4
        wk_sub = {}      # c -> (tile, col_base)
        wk_splits = [(0, 2), (2, 2), (4, 4), (8, 4), (12, 4)]
        for si, (c0, ncs) in enumerate(wk_splits):
            wkt = wkp.tile([128, ncs * 512], BF16, tag=f"wk{si}",
                           name=f"wk_sub{si}", bufs=1)
            nc.gpsimd.dma_start(wkt[:], wk[:, c0 * 512:(c0 + ncs) * 512])
            for c in range(c0, c0 + ncs):
                wk_sub[c] = (wkt, (c - c0) * 512)
        wv_sub = []
        for q4 in range(4):
            wvt = wvp.tile([128, 4 * 512], BF16, tag="wv", name=f"wv_sub{q4}",
                           bufs=4)
            nc.gpsimd.dma_start(wvt[:], wv[:, q4 * 2048:(q4 + 1) * 2048])
            wv_sub.append(wvt)

        def wk_sl(c, g):
            t, base = wk_sub[c]
            return t[:, base + g * 128:base + (g + 1) * 128]

        def wv_sl(c):
            return wv_sub[c // 4][:, (c % 4) * 512:(c % 4 + 1) * 512]

        # ---- phase A: K^T (RoPE'd) and V over 3 spans of 512 kv slots
        kt_sb = [ktp.tile([128, TKV], BF16, tag="kt", name=f"kt{g}")
                 for g in range(KVH)]
        v_sb = [vp.tile([128, 512], BF16, tag="v", name=f"v{m}")
                for m in range(NMT)]

        consts_loaded = [False]
        const_sb = {}

        def cload(ap, shape, dtype, tag):
            t = consts.tile(shape, dtype, tag=tag, name=tag)
            nc.sync.dma_start(t[:], ap[:])
            return t

        for s in range(3):
            xs = []
            for c in range(NCC):
                xt = xsp.tile([128, 512], BF16, tag="xs", name=f"xs{s}_{c}")
                nc.sync.dma_start(
                    xt[:], xkvT[s * DIM + c * 128:s * DIM + (c + 1) * 128, :])
                xs.append(xt)
            cosk_s = ropet.tile([128, 512], F32, tag="rt", name=f"cosk{s}")
            nc.sync.dma_start(cosk_s[:], cosk[s * 128:(s + 1) * 128, :])
            sink_s = ropet.tile([128, 512], F32, tag="rt", name=f"sink{s}")
            nc.sync.dma_start(sink_s[:], sink[s * 128:(s + 1) * 128, :])
            if not consts_loaded[0]:
                const_sb["rotp"] = cload(rotp, [128, 128], BF16, "rotp")
                const_sb["ones"] = cload(ones, [128, 128], BF16, "ones")
                const_sb["kbias"] = cload(kbias, [128, NMT], F32, "kbias")
                const_sb["valid"] = cload(valid, [128, 384], BF16, "valid")
                const_sb["maskB"] = cload(mask_dram["maskB"], [128, 128],
                                          BF16, "maskB")
                const_sb["maskA"] = cload(mask_dram["maskA"], [128, 128],
                                          BF16, "maskA")
                consts_loaded[0] = True

            # K^T projection: 4 chains (one per kv head) across acc banks.
            # g=3's first writes are deferred 12 matmuls so the WAR on last
            # span's kps[3] (read late by its rope t2-mul) is hidden.
            kps = [ps_acc.tile([128, 512], F32, tag="acc", name=f"kps{s}_{g}")
                   for g in range(KVH)]

            def kmm(c, g):
                nc.tensor.matmul(kps[g][:], wk_sl(c, g), xs[c][:],
                                 start=(c == 0), stop=(c == NCC - 1))

            for c in range(4):
                for g in range(3):
                    kmm(c, g)
            for c in range(4):
                kmm(c, 3)
            for c in range(4, NCC):
                for g in range(KVH):
                    kmm(c, g)

            # rope rotate-half sources, copied early on ACT
            ssb = []
            for g in range(KVH):
                sg = tmp.tile([128, 512], BF16, tag="ssb", name=f"ssb{s}_{g}")
                nc.scalar.activation(sg[:], kps[g][:], Copy)
                ssb.append(sg)

            def ropek(g):
                r_ps = ps_r.tile([128, 512], F32, tag="rp", name=f"rk{s}_{g}")
                nc.tensor.matmul(r_ps[:], const_sb["rotp"][:], ssb[g][:],
                                 start=True, stop=True)
                t1 = t12.tile([128, 512], F32, tag="t12", name=f"kt1_{s}_{g}")
                nc.vector.tensor_mul(t1[:], r_ps[:], sink_s[:])
                t2 = t12.tile([128, 512], F32, tag="t12", name=f"kt2_{s}_{g}")
                nc.vector.tensor_mul(t2[:], kps[g][:], cosk_s[:])
                nc.vector.tensor_add(kt_sb[g][:, s * 512:(s + 1) * 512],
                                     t1[:], t2[:])

            # V projection (natural layout) in 2 passes of 2 PSUM banks,
            # with the 4 rope matmuls interleaved between V-chain batches
            vps = {}

            def vchain(tts, c0, c1):
                for c in range(c0, c1):
                    for tt in tts:
                        nc.tensor.matmul(
                            vps[tt][:],
                            xs[c][:, tt * 128:(tt + 1) * 128],
                            wv_sl(c),
                            start=(c == 0), stop=(c == NCC - 1))

            for tt in (0, 1):
                vps[tt] = ps_s.tile([128, 512], F32, tag="sps",
                                    name=f"vps{s}_{tt}")
            vchain((0, 1), 0, 8)
            ropek(0)
            vchain((0, 1), 8, NCC)
            ropek(1)
            for tt in (0, 1):
                nc.scalar.activation(v_sb[4 * s + tt][:], vps[tt][:], Copy)
            for tt in (2, 3):
                vps[tt] = ps_s.tile([128, 512], F32, tag="sps",
                                    name=f"vps{s}_{tt}")
            vchain((2, 3), 0, 4)
            ropek(2)
            vchain((2, 3), 4, 8)
            ropek(3)
            vchain((2, 3), 8, NCC)
            for tt in (2, 3):
                nc.scalar.activation(v_sb[4 * s + tt][:], vps[tt][:], Copy)

        # ---- phase A2: all 16 Q^T heads projected + RoPE'd.
        # Rope matmuls of pair p are emitted mid-chain of pair p+1 so the
        # PE never waits on the ACT rotate-source copy.
        xq_all = xqp.tile([128, NCC * 512], BF16, tag="xq", name="xq_all")
        nc.sync.dma_start(xq_all[:], xq[:])
        cosq_sb = ropet.tile([128, 512], F32, tag="rt", name="cosq")
        nc.sync.dma_start(cosq_sb[:], cosq[:])
        sinq_sb = ropet.tile([128, 512], F32, tag="rt", name="sinq")
        nc.sync.dma_start(sinq_sb[:], sinq[:])

        qts = {}

        def ropeq(p_, j, qpair):
            sg = tmp.tile([128, 512], BF16, tag="ssb", name=f"sq{p_}_{j}")
            nc.scalar.activation(sg[:], qpair[j][:], Copy)
            r_ps = ps_s.tile([128, 512], F32, tag="sps", name=f"rq{p_}_{j}")
            nc.tensor.matmul(r_ps[:], const_sb["rotp"][:], sg[:],
                             start=True, stop=True)
            t1 = t12.tile([128, 512], F32, tag="t12", name=f"qt1_{p_}_{j}")
            nc.vector.tensor_mul(t1[:], r_ps[:], sinq_sb[:])
            t2 = t12.tile([128, 512], F32, tag="t12", name=f"qt2_{p_}_{j}")
            nc.vector.tensor_mul(t2[:], qpair[j][:], cosq_sb[:])
            qtj = qtp.tile([128, 512], BF16, tag="qt", name=f"qt{2 * p_ + j}")
            nc.vector.tensor_add(qtj[:], t1[:], t2[:])
            qts[2 * p_ + j] = qtj

        prev = None  # (p_, qpair) whose ropes are pending
        for p_ in range(H // 2):
            qpair = [ps_acc.tile([128, 512], F32, tag="acc",
                                 name=f"qps{p_}_{j}") for j in range(2)]
            # sync queue: sits behind all xs tiles, so these 1MB transfers
            # cannot starve phase A's time-critical loads
            wqt = wqp.tile([128, 4096], BF16, tag="wq", name=f"wqt{p_}")
            nc.sync.dma_start(wqt[:], wq[p_ * 128:(p_ + 1) * 128, :])
            for c in range(NCC):
                if c == 6 and prev is not None:
                    ropeq(prev[0], 0, prev[1])
                if c == 10 and prev is not None:
                    ropeq(prev[0], 1, prev[1])
                    prev = None
                for j in range(2):
                    nc.tensor.matmul(qpair[j][:],
                                     wqt[:, c * 256 + j * 128:
                                         c * 256 + (j + 1) * 128],
                                     xq_all[:, c * 512:(c + 1) * 512],
                                     start=(c == 0), stop=(c == NCC - 1))
            prev = (p_, qpair)
        ropeq(prev[0], 0, prev[1])
        ropeq(prev[0], 1, prev[1])

        # ---- phase B: attention, software-pipelined per head
        yt_sb = [ytp.tile([128, TQ], BF16, tag="yt", name=f"yt{h}")
                 for h in range(H)]

        ucount = [0]
        for h in range(H):
            g = h // GQ
            qt = qts[h]
            acc_y = ps_acc.tile([128, TQ], F32, tag="acc", name=f"yps{h}")
            acc_d = ps_acc.tile([128, TQ], F32, tag="acc", name=f"dps{h}")
            p_l = {}

            def qk(ui, h=h, g=g, qt=qt, p_l=p_l):
                unit = UNITS[ui]
                merged = len(unit) > 1
                # rotate S banks over ps_s (3) + the A-phase rope bank (1)
                u = ucount[0]
                ucount[0] += 1
                pool = ps_r if u % 4 == 3 else ps_s
                tagn = "rp" if u % 4 == 3 else "sps"
                sps = pool.tile([128, 512], F32, tag=tagn,
                                name=f"sps{h}_{unit[0]}")
                ext = 0
                for m in unit:
                    qlo, qhi = SPANS[m]
                    w = qhi - qlo
                    off = UOFF[m] if merged else 0
                    nc.tensor.matmul(sps[:, off:off + w],
                                     kt_sb[g][:, m * 128:(m + 1) * 128],
                                     qt[:, qlo:qhi], start=True, stop=True)
                    ext = max(ext, off + w)
                p = pp.tile([128, 512], BF16, tag="p", name=f"p{h}_{unit[0]}")
                bias = 0.0 if merged else const_sb["kbias"][:, unit[0]:
                                                            unit[0] + 1]
                nc.scalar.activation(p[:, :ext], sps[:, :ext], Exp,
                                     bias=bias, scale=SCALE)
                for m in unit:
                    mk = MASKS[m]
                    if mk is not None:
                        qlo, qhi = SPANS[m]
                        off = (UOFF[m] if merged else 0) - qlo
                        name_, lo, hi = mk
                        nc.vector.tensor_mul(p[:, lo + off:hi + off],
                                             p[:, lo + off:hi + off],
                                             const_sb[name_][:])
                p_l[ui] = p

            def pv(ui, h=h, g=g, acc_y=acc_y, acc_d=acc_d, p_l=p_l):
                unit = UNITS[ui]
                merged = len(unit) > 1
                p = p_l.pop(ui)
                first = ui == 0
                last_unit = ui == len(UNITS) - 1
                for m in unit:
                    qlo, qhi = SPANS[m]
                    w = qhi - qlo
                    off = UOFF[m] if merged else 0
                    last = last_unit and m == unit[-1]
                    if m in MERGED:
                        vi = MERGED[m]
                        den_st = const_sb["valid"][:, vi * 128:(vi + 1) * 128]
                    else:
                        den_st = const_sb["ones"][:]
                    nc.tensor.matmul(acc_y[:, qlo:qhi],
                                     v_sb[m][:, g * 128:(g + 1) * 128],
                                     p[:, off:off + w], start=first,
                                     stop=last)
                    nc.tensor.matmul(acc_d[:, qlo:qhi], den_st,
                                     p[:, off:off + w], start=first,
                                     stop=last)
                    first = False

            for i in range(LOOK):
                qk(i)
            for i in range(len(UNITS)):
                if i + LOOK < len(UNITS):
                    qk(i + LOOK)
                pv(i)

            rcp = fin.tile([128, TQ], F32, tag="rcp", name=f"rcp{h}")
            nc.vector.reciprocal_approx_fast(rcp[:], acc_d[:])
            nc.vector.tensor_mul(yt_sb[h][:], acc_y[:], rcp[:])

        # ---- phase C: O^T projection in e-tile pairs; wo streamed as
        # 8KB-row pair tiles (prefetch depth = wop bufs via queue ordering)
        for n0 in range(0, NCC, 2):
            np_ = n0 // 2
            wot = wop.tile([128, 4096], BF16, tag="wo", name=f"wot{np_}")
            nc.sync.dma_start(wot[:], wo[np_ * 128:(np_ + 1) * 128, :])
            opair = [ps_acc.tile([128, 512], F32, tag="acc",
                                 name=f"ops{n0}_{j}") for j in range(2)]
            for h in range(H):
                for j in range(2):
                    nc.tensor.matmul(opair[j][:],
                                     wot[:, h * 256 + j * 128:
                                         h * 256 + (j + 1) * 128],
                                     yt_sb[h][:],
                                     start=(h == 0), stop=(h == H - 1))
            for j in range(2):
                osb = fin.tile([128, TQ], F32, tag="osb", name=f"osb{n0}_{j}")
                nc.scalar.activation(osb[:], opair[j][:], Copy)
                nc.sync.dma_start(outT[(n0 + j) * 128:(n0 + j + 1) * 128, :],
                                  osb[:])


# ---------------------------------------------------------------- host side
def _host_inputs(x, Wq, Wk, Wv, Wo):
    x = np.asarray(x, dtype=np.float32).reshape(T, DIM)

    inv_freq = 1.0 / (ROPE_BASE ** (np.arange(0, D, 2, dtype=np.float64) / D))
    dfreq = np.concatenate([inv_freq, inv_freq])  # [128] per-dim freq

    # fat-row packed weight layouts (see _build comments)
    wq_r = np.ascontiguousarray(
        np.asarray(Wq).reshape(NCC, 128, 8, 256).transpose(2, 1, 0, 3)
        .reshape(8 * 128, 4096)).astype(BF)
    wo_r = np.ascontiguousarray(
        np.asarray(Wo).reshape(H, 128, 8, 256).transpose(2, 1, 0, 3)
        .reshape(8 * 128, 4096)).astype(BF)
    wk_r = np.ascontiguousarray(
        np.asarray(Wk, np.float32).reshape(NCC, 128, 512).transpose(1, 0, 2)
        .reshape(128, NCC * 512)).astype(BF)
    wv_r = np.ascontiguousarray(
        np.asarray(Wv, np.float32).reshape(NCC, 128, 512).transpose(1, 0, 2)
        .reshape(128, NCC * 512)).astype(BF)

    u = np.arange(128)[:, None]
    maskB = (np.arange(128)[None, :] < u).astype(BF)        # qq>=u -> 0
    maskA = (u <= np.arange(128)[None, :]).astype(BF)       # u>qq -> 0

    rotp = np.zeros((128, 128), np.float32)
    d = np.arange(128)
    rotp[(d + 64) % 128, d] = 1.0  # out[d] = in[(d+64)%128]
    rotp = rotp.astype(BF)

    ones = np.ones((128, 128), BF)

    in_maps = []
    for c in range(N_CORES):
        qs = c * TQ
        xqc = x[qs:qs + TQ]                     # [512, 2048]
        xkv = np.zeros((TKV, DIM), np.float32)  # [1536, 2048]
        lo = qs - WIN
        src_lo = max(0, lo)
        xkv[src_lo - lo:TKV] = x[src_lo:qs + TQ]

        pos_q = np.arange(qs, qs + TQ, dtype=np.float64)
        pos_k = np.arange(lo, qs + TQ, dtype=np.float64)
        angq = dfreq[:, None] * pos_q[None, :]  # [128, 512]
        angk = dfreq[:, None] * pos_k[None, :]  # [128, 1536]
        sgn = np.where(np.arange(D) < D // 2, -1.0, 1.0)[:, None]

        kb = np.zeros((128, NMT), np.float32)
        for m in range(NMT):
            t_abs = 128 * m + np.arange(128)
            kb[:, m] = np.where(t_abs < WIN - qs, -30.0, 0.0)
        # valid-vector den stationaries for the merged exp tiles {0,1,2,11}:
        # stat[t, i] = 1 unless kv slot 128m+t is left-padding on this core
        vld = np.zeros((128, 384), np.float32)
        for vi, m in enumerate((0, 1, 2)):
            t_abs = 128 * m + np.arange(128)
            vld[:, vi * 128:(vi + 1) * 128] = np.where(
                t_abs < WIN - qs, 0.0, 1.0)[:, None]

        in_maps.append({
            "xq": np.ascontiguousarray(
                xqc.T.reshape(NCC, 128, TQ).transpose(1, 0, 2)
                .reshape(128, NCC * TQ)).astype(BF),
            "xkvT": np.ascontiguousarray(
                xkv.T.reshape(DIM, 3, 512).transpose(1, 0, 2)
                .reshape(3 * DIM, 512)).astype(BF),
            "wq": wq_r, "wk": wk_r, "wv": wv_r, "wo": wo_r,
            "cosq": np.cos(angq).astype(np.float32),
            "sinq": (sgn * np.sin(angq)).astype(np.float32),
            "cosk": np.ascontiguousarray(np.cos(angk).astype(np.float32)
                .reshape(D, 3, 512).transpose(1, 0, 2)).reshape(3 * D, 512),
            "sink": np.ascontiguousarray(((sgn * np.sin(angk)).astype(np.float32))
                .reshape(D, 3, 512).transpose(1, 0, 2)).reshape(3 * D, 512),
            "kbias": kb,
            "valid": vld.astype(BF),
            "maskB": maskB, "maskA": maskA,
            "rotp": rotp,
            "ones": ones,
        })
    return in_maps


def kernel(x, Wq, Wk, Wv, Wo, _trace=False, _trace_kwargs=None):
    nc = _build()
    in_maps = _host_inputs(x, Wq, Wk, Wv, Wo)
    res = run_bass_kernel_spmd(nc, in_maps, core_ids=list(range(N_CORES)),
                               trace=_trace, **(_trace_kwargs or {}))
    out = np.empty((1, T, DIM), np.float32)
    for c in range(N_CORES):
        out[0, c * TQ:(c + 1) * TQ, :] = res.results[c]["outT"].T
    if _trace:
        kernel.last_results = res
    return out


# revision 23
# speedup vs baseline: 1.7469x; 1.0927x over previous
"""Sliding-window GQA attention (T=4096, DIM=2048, H=16, KVH=4, D=128, W=1024)
as an 8-core SPMD Trainium2 Bass/Tile kernel.

Sharding (v7): 4-way sequence x 2-way head. Core c covers queries
[1024*(c%4), +1024) for heads [8*(c//4), +8). The 1024-slot K/V halo is
amortized over 2x the queries per core (halo recompute drops from 2x to 1x
of owned work, -28us PE/core vs 8-way sequence). The two cores sharing a
q-range emit partial outputs (linear in heads); the host sums them.

Phases (attention keeps the same tuned 16-virtual-head x 12-tile structure:
8 local heads x 2 q-blocks of 512):
  A : K^T (RoPE'd, bf16) and V (natural, bf16) over 4 spans of 512 kv slots
  A2: 8 local Q^T head-halves projected + RoPE'd (chains reuse the resident
      own-x span tiles; rope matmuls software-pipelined one unit behind)
  B : attention; S(unit+3) issued before PV(unit); merged full-width exp
      units; valid-vector denominator on left-padding tiles
  C : partial O^T projection, bf16 out
All matmul operands bf16 (PSUM fp32), fat-row DMA layouts, exact spans.
"""

import math
import os
import sys

import numpy as np


def _ensure_paths():
    for p in (
        "/root/.axon_site",
        "/root/.axon_site/_ro/trn_rl_repo",
        "/root/.axon_site/_ro/pypackages",
        "/opt/trn_rl_repo",
        "/opt/pypackages",
    ):
        if os.path.isdir(p) and p not in sys.path:
            sys.path.append(p)


try:
    import concourse.bass as bass  # noqa: F401
except ImportError:
    _ensure_paths()

import ml_dtypes

import concourse.bass as bass
import concourse.mybir as mybir
import concourse.tile as tile
from concourse import bacc
from concourse.bass_utils import run_bass_kernel_spmd

# ---------------------------------------------------------------- constants
N_CORES = 8
SEQ_SH = 4                 # sequence shards
T = 4096
DIM = 2048
H = 16
KVH = 4
D = 128
WIN = 1024
ROPE_BASE = 10000.0

TQ = T // SEQ_SH           # 1024 queries per core
QB = TQ // 512             # 2 q-blocks of 512
TKV = TQ + WIN             # 2048 kv slots per core
NSP = TKV // 512           # 4 kv spans of 512
NMT = 12                   # kv tiles per (head, q-block) window of 1536
NCC = DIM // 128           # 16 contraction chunks
SCALE = 1.0 / math.sqrt(D)
LH = H // (N_CORES // SEQ_SH)   # 8 local heads per core
LKV = LH // 4              # 2 local kv groups

F32 = mybir.dt.float32
BF16 = mybir.dt.bfloat16
BF = ml_dtypes.bfloat16

# per kv-tile m: exact (qlo, qhi) span of q (within its 512 q-block)
SPANS = {
    0: (0, 128), 1: (0, 256), 2: (0, 384), 3: (0, 512),
    4: (0, 512), 5: (0, 512), 6: (0, 512), 7: (0, 512),
    8: (0, 512), 9: (128, 512), 10: (256, 512), 11: (384, 512),
}
MASKS = {
    0: ("maskB", 0, 128), 1: ("maskB", 128, 256),
    2: ("maskB", 256, 384), 3: ("maskB", 384, 512),
    4: None, 5: None, 6: None, 7: None,
    8: ("maskA", 0, 128), 9: ("maskA", 128, 256),
    10: ("maskA", 256, 384), 11: ("maskA", 384, 512),
}
# Pipeline units: one PSUM bank + one exp each, all full 512-wide.
# {4} first (start=True clears the bank), {2,11} last (stop on 11).
UNITS = [(4,), (5,), (6,), (7,), (9, 0), (10, 1), (3,), (8,), (2, 11)]
UOFF = {9: 0, 0: 384, 10: 0, 1: 256, 2: 0, 11: 384}  # col offset in unit bank
MERGED = {0: 0, 1: 1, 2: 2}  # m -> valid-table block (m9/10/11 never padded)
LOOK = 3                   # S-unit lookahead depth in phase B


# ---------------------------------------------------------------- device code
_NC_CACHE = None


def _build():
    global _NC_CACHE
    if _NC_CACHE is not None:
        return _NC_CACHE

    nc = bacc.Bacc("TRN2", target_bir_lowering=False, debug=False,
                   num_devices=N_CORES)

    # DRAM I/O (per-core contents via in_maps). Fat-row packed layouts:
    #   xkvT[s*2048 + c*128 + r, t]  = x^T chunk, span s, slot t
    #   wq[p*128+r, c*256+jc] = Wq[c*128+r, (hh*8+2p)*128 ... ]  (8KB rows)
    #   wk[r, c*256+gc]       = Wk[c*128+r, hh*256+gc]           (8KB rows)
    #   wo[np*128+r, hl*256+jc] = Wo[(hh*8+hl)*128+r, np*256+jc] (4KB rows)
    xkvT = nc.dram_tensor("xkvT", [NSP * DIM, 512], BF16,
                          kind="ExternalInput").ap()
    wq = nc.dram_tensor("wq", [4 * 128, 4096], BF16, kind="ExternalInput").ap()
    wk = nc.dram_tensor("wk", [128, NCC * 256], BF16, kind="ExternalInput").ap()
    wv = nc.dram_tensor("wv", [128, NCC * 256], BF16, kind="ExternalInput").ap()
    wo = nc.dram_tensor("wo", [8 * 128, 2048], BF16, kind="ExternalInput").ap()
    cosq = nc.dram_tensor("cosq", [D, TQ], F32, kind="ExternalInput").ap()
    sinq = nc.dram_tensor("sinq", [D, TQ], F32, kind="ExternalInput").ap()
    cosk = nc.dram_tensor("cosk", [NSP * D, 512], F32, kind="ExternalInput").ap()
    sink = nc.dram_tensor("sink", [NSP * D, 512], F32, kind="ExternalInput").ap()
    kbias = nc.dram_tensor("kbias", [128, QB * NMT], F32,
                           kind="ExternalInput").ap()
    valid = nc.dram_tensor("valid", [128, QB * 384], BF16,
                           kind="ExternalInput").ap()
    maskB = nc.dram_tensor("maskB", [128, 128], BF16, kind="ExternalInput").ap()
    maskA = nc.dram_tensor("maskA", [128, 128], BF16, kind="ExternalInput").ap()
    rotp = nc.dram_tensor("rotp", [128, 128], BF16, kind="ExternalInput").ap()
    ones = nc.dram_tensor("ones", [128, 128], BF16, kind="ExternalInput").ap()
    outT = nc.dram_tensor("outT", [DIM, TQ], BF16, kind="ExternalOutput").ap()

    mask_dram = {"maskB": maskB, "maskA": maskA}

    with tile.TileContext(nc) as tc:
        _emit(nc, tc, xkvT, wq, wk, wv, wo, cosq, sinq, cosk, sink,
              kbias, valid, mask_dram, rotp, ones, outT)

    nc.compile()
    _NC_CACHE = nc
    return nc


def _emit(nc, tc, xkvT, wq, wk, wv, wo, cosq, sinq, cosk, sink,
          kbias, valid, mask_dram, rotp, ones, outT):
    from contextlib import ExitStack

    ctx = ExitStack()
    with ctx:
        # SBUF pools (~190KB/partition)
        consts = ctx.enter_context(tc.tile_pool(name="consts", bufs=1))
        ropet = ctx.enter_context(tc.tile_pool(name="ropet", bufs=4))
        xsp = ctx.enter_context(tc.tile_pool(name="xsp", bufs=3 * NCC))
        wkp = ctx.enter_context(tc.tile_pool(name="wkp", bufs=1))
        wvp = ctx.enter_context(tc.tile_pool(name="wvp", bufs=1))
        wqp = ctx.enter_context(tc.tile_pool(name="wqp", bufs=4))
        wop = ctx.enter_context(tc.tile_pool(name="wop", bufs=3))
        qtp = ctx.enter_context(tc.tile_pool(name="qtp", bufs=LH))
        ktp = ctx.enter_context(tc.tile_pool(name="ktp", bufs=LKV))
        vp = ctx.enter_context(tc.tile_pool(name="vp", bufs=QB * NMT - 8))
        ytp = ctx.enter_context(tc.tile_pool(name="ytp", bufs=LH))
        pp = ctx.enter_context(tc.tile_pool(name="pp", bufs=5))
        tmp = ctx.enter_context(tc.tile_pool(name="tmp", bufs=4))
        t12 = ctx.enter_context(tc.tile_pool(name="t12", bufs=4))
        fin = ctx.enter_context(tc.tile_pool(name="fin", bufs=2))
        # PSUM: exactly 8 banks
        ps_acc = ctx.enter_context(tc.tile_pool(name="ps_acc", bufs=4,
                                                space="PSUM"))
        ps_s = ctx.enter_context(tc.tile_pool(name="ps_s", bufs=3,
                                              space="PSUM"))
        ps_r = ctx.enter_context(tc.tile_pool(name="ps_r", bufs=1,
                                              space="PSUM"))

        Exp = mybir.ActivationFunctionType.Exp
        Copy = mybir.ActivationFunctionType.Copy

        # ---- persistent weights (gpsimd queue); wk split for early start
        wk_sub = {}      # c -> (tile, col_base)
        wk_splits = [(0, 2), (2, 2), (4, 4), (8, 8)]
        for si, (c0, ncs) in enumerate(wk_splits):
            wkt = wkp.tile([128, ncs * 256], BF16, tag=f"wk{si}",
                           name=f"wk_sub{si}", bufs=1)
            nc.gpsimd.dma_start(wkt[:], wk[:, c0 * 256:(c0 + ncs) * 256])
            for c in range(c0, c0 + ncs):
                wk_sub[c] = (wkt, (c - c0) * 256)
        wv_sub = []
        for q2 in range(2):
            wvt = wvp.tile([128, 8 * 256], BF16, tag="wv", name=f"wv_sub{q2}",
                           bufs=2)
            nc.gpsimd.dma_start(wvt[:], wv[:, q2 * 2048:(q2 + 1) * 2048])
            wv_sub.append(wvt)

        def wk_sl(c, g):
            t, base = wk_sub[c]
            return t[:, base + g * 128:base + (g + 1) * 128]

        def wv_sl(c):
            return wv_sub[c // 8][:, (c % 8) * 256:(c % 8 + 1) * 256]

        # ---- phase A: K^T (RoPE'd) and V over NSP spans of 512 kv slots
        kt_sb = [ktp.tile([128, TKV], BF16, tag="kt", name=f"kt{g}")
                 for g in range(LKV)]
        v_sb = [vp.tile([128, 256], BF16, tag="v", name=f"v{mt}",
                        bufs=QB * NMT - 8)
                for mt in range(QB * NMT - 8)]   # 16 slot-tiles x (2g*128)

        consts_loaded = [False]
        const_sb = {}

        def cload(ap, shape, dtype, tag):
            t = consts.tile(shape, dtype, tag=tag, name=tag)
            nc.sync.dma_start(t[:], ap[:])
            return t

        xs_all = []
        for s in range(NSP):
            xs = []
            for c in range(NCC):
                xt = xsp.tile([128, 512], BF16, tag="xs", name=f"xs{s}_{c}")
                nc.sync.dma_start(
                    xt[:], xkvT[s * DIM + c * 128:s * DIM + (c + 1) * 128, :])
                xs.append(xt)
            xs_all.append(xs)
            cosk_s = ropet.tile([128, 512], F32, tag="rt", name=f"cosk{s}")
            nc.sync.dma_start(cosk_s[:], cosk[s * 128:(s + 1) * 128, :])
            sink_s = ropet.tile([128, 512], F32, tag="rt", name=f"sink{s}")
            nc.sync.dma_start(sink_s[:], sink[s * 128:(s + 1) * 128, :])
            if not consts_loaded[0]:
                const_sb["rotp"] = cload(rotp, [128, 128], BF16, "rotp")
                const_sb["ones"] = cload(ones, [128, 128], BF16, "ones")
                const_sb["kbias"] = cload(kbias, [128, QB * NMT], F32, "kbias")
                const_sb["valid"] = cload(valid, [128, QB * 384], BF16,
                                          "valid")
                const_sb["maskB"] = cload(mask_dram["maskB"], [128, 128],
                                          BF16, "maskB")
                const_sb["maskA"] = cload(mask_dram["maskA"], [128, 128],
                                          BF16, "maskA")
                for qb in range(QB):
                    const_sb[f"cosq{qb}"] = cload(
                        cosq[:, qb * 512:(qb + 1) * 512], [128, 512], F32,
                        f"cosq{qb}")
                    const_sb[f"sinq{qb}"] = cload(
                        sinq[:, qb * 512:(qb + 1) * 512], [128, 512], F32,
                        f"sinq{qb}")
                consts_loaded[0] = True

            # K^T projection: LKV chains; second chain's first writes
            # deferred so last span's kps WAR (rope t2-mul) is hidden
            kps = [ps_acc.tile([128, 512], F32, tag="acc", name=f"kps{s}_{g}")
                   for g in range(LKV)]

            def kmm(c, g):
                nc.tensor.matmul(kps[g][:], wk_sl(c, g), xs[c][:],
                                 start=(c == 0), stop=(c == NCC - 1))

            for c in range(4):
                kmm(c, 0)
            for c in range(4):
                kmm(c, 1)
            for c in range(4, NCC):
                for g in range(LKV):
                    kmm(c, g)

            ssb = []
            for g in range(LKV):
                sg = tmp.tile([128, 512], BF16, tag="ssb", name=f"ssb{s}_{g}")
                nc.scalar.activation(sg[:], kps[g][:], Copy)
                ssb.append(sg)

            def ropek(g):
                r_ps = ps_r.tile([128, 512], F32, tag="rp", name=f"rk{s}_{g}")
                nc.tensor.matmul(r_ps[:], const_sb["rotp"][:], ssb[g][:],
                                 start=True, stop=True)
                t1 = t12.tile([128, 512], F32, tag="t12", name=f"kt1_{s}_{g}")
                nc.vector.tensor_mul(t1[:], r_ps[:], sink_s[:])
                t2 = t12.tile([128, 512], F32, tag="t12", name=f"kt2_{s}_{g}")
                nc.vector.tensor_mul(t2[:], kps[g][:], cosk_s[:])
                nc.vector.tensor_add(kt_sb[g][:, s * 512:(s + 1) * 512],
                                     t1[:], t2[:])

            # V projection (natural layout, both groups as 256-wide rhs),
            # 2 passes of 2 banks, rope matmuls interleaved
            vps = {}

            def vchain(tts, c0, c1):
                for c in range(c0, c1):
                    for tt in tts:
                        nc.tensor.matmul(
                            vps[tt][:],
                            xs[c][:, tt * 128:(tt + 1) * 128],
                            wv_sl(c),
                            start=(c == 0), stop=(c == NCC - 1))

            for tt in (0, 1):
                vps[tt] = ps_s.tile([128, 256], F32, tag="sps",
                                    name=f"vps{s}_{tt}")
            vchain((0, 1), 0, 8)
            ropek(0)
            vchain((0, 1), 8, NCC)
            ropek(1)
            for tt in (0, 1):
                nc.scalar.activation(v_sb[4 * s + tt][:], vps[tt][:], Copy)
            for tt in (2, 3):
                vps[tt] = ps_s.tile([128, 256], F32, tag="sps",
                                    name=f"vps{s}_{tt}")
            vchain((2, 3), 0, NCC)
            for tt in (2, 3):
                nc.scalar.activation(v_sb[4 * s + tt][:], vps[tt][:], Copy)

        # ---- phase A2: 8 local Q^T heads projected + RoPE'd. Chains reuse
        # the resident own-x span tiles (spans 2,3 <-> q-blocks 0,1); rope
        # matmuls of a unit are emitted mid-chain of the next unit.
        qts = [qtp.tile([128, TQ], BF16, tag="qt", name=f"qt{hl}")
               for hl in range(LH)]

        def ropeq(p_, qb, j, qpair):
            sg = tmp.tile([128, 512], BF16, tag="ssb", name=f"sq{p_}{qb}{j}")
            nc.scalar.activation(sg[:], qpair[j][:], Copy)
            r_ps = ps_s.tile([128, 512], F32, tag="sps", name=f"rq{p_}{qb}{j}")
            nc.tensor.matmul(r_ps[:], const_sb["rotp"][:], sg[:],
                             start=True, stop=True)
            t1 = t12.tile([128, 512], F32, tag="t12", name=f"qt1_{p_}{qb}{j}")
            nc.vector.tensor_mul(t1[:], r_ps[:], const_sb[f"sinq{qb}"][:])
            t2 = t12.tile([128, 512], F32, tag="t12", name=f"qt2_{p_}{qb}{j}")
            nc.vector.tensor_mul(t2[:], qpair[j][:], const_sb[f"cosq{qb}"][:])
            nc.vector.tensor_add(qts[2 * p_ + j][:, qb * 512:(qb + 1) * 512],
                                 t1[:], t2[:])

        wqts = {}
        prev = None  # (p_, qb, qpair) whose ropes are pending
        for p_ in range(LH // 2):
            wqt = wqp.tile([128, 4096], BF16, tag="wq", name=f"wqt{p_}")
            nc.sync.dma_start(wqt[:], wq[p_ * 128:(p_ + 1) * 128, :])
            wqts[p_] = wqt
        for p_ in range(LH // 2):
            wqt = wqts[p_]
            for qb in range(QB):
                qpair = [ps_acc.tile([128, 512], F32, tag="acc",
                                     name=f"qps{p_}{qb}{j}")
                         for j in range(2)]
                xsq = xs_all[2 + qb]
                for c in range(NCC):
                    if c == 6 and prev is not None:
                        ropeq(prev[0], prev[1], 0, prev[2])
                    if c == 10 and prev is not None:
                        ropeq(prev[0], prev[1], 1, prev[2])
                        prev = None
                    for j in range(2):
                        nc.tensor.matmul(qpair[j][:],
                                         wqt[:, c * 256 + j * 128:
                                             c * 256 + (j + 1) * 128],
                                         xsq[c][:],
                                         start=(c == 0), stop=(c == NCC - 1))
                prev = (p_, qb, qpair)
        ropeq(prev[0], prev[1], 0, prev[2])
        ropeq(prev[0], prev[1], 1, prev[2])

        # ---- phase B: attention over 16 virtual heads (hl, qb)
        yt_sb = [ytp.tile([128, TQ], BF16, tag="yt", name=f"yt{hl}")
                 for hl in range(LH)]

        ucount = [0]
        for hl in range(LH):
            for qb in range(QB):
                gl = hl // 4
                vh = f"{hl}_{qb}"
                acc_y = ps_acc.tile([128, 512], F32, tag="acc",
                                    name=f"yps{vh}")
                acc_d = ps_acc.tile([128, 512], F32, tag="acc",
                                    name=f"dps{vh}")
                p_l = {}

                def qk(ui, gl=gl, qb=qb, vh=vh, hl=hl, p_l=p_l):
                    unit = UNITS[ui]
                    merged = len(unit) > 1
                    u = ucount[0]
                    ucount[0] += 1
                    pool = ps_r if u % 4 == 3 else ps_s
                    tagn = "rp" if u % 4 == 3 else "sps"
                    sps = pool.tile([128, 512], F32, tag=tagn,
                                    name=f"sps{vh}_{unit[0]}")
                    ext = 0
                    for m in unit:
                        qlo, qhi = SPANS[m]
                        w = qhi - qlo
                        off = UOFF[m] if merged else 0
                        nc.tensor.matmul(
                            sps[:, off:off + w],
                            kt_sb[gl][:, qb * 512 + m * 128:
                                      qb * 512 + (m + 1) * 128],
                            qts[hl][:, qb * 512 + qlo:qb * 512 + qhi],
                            start=True, stop=True)
                        ext = max(ext, off + w)
                    p = pp.tile([128, 512], BF16, tag="p",
                                name=f"p{vh}_{unit[0]}")
                    bias = 0.0 if merged else \
                        const_sb["kbias"][:, qb * NMT + unit[0]:
                                          qb * NMT + unit[0] + 1]
                    nc.scalar.activation(p[:, :ext], sps[:, :ext], Exp,
                                         bias=bias, scale=SCALE)
                    for m in unit:
                        mk = MASKS[m]
                        if mk is not None:
                            qlo, qhi = SPANS[m]
                            off = (UOFF[m] if merged else 0) - qlo
                            name_, lo, hi = mk
                            nc.vector.tensor_mul(p[:, lo + off:hi + off],
                                                 p[:, lo + off:hi + off],
                                                 const_sb[name_][:])
                    p_l[ui] = p

                def pv(ui, gl=gl, qb=qb, acc_y=acc_y, acc_d=acc_d, p_l=p_l):
                    unit = UNITS[ui]
                    merged = len(unit) > 1
                    p = p_l.pop(ui)
                    first = ui == 0
                    last_unit = ui == len(UNITS) - 1
                    for m in unit:
                        qlo, qhi = SPANS[m]
                        w = qhi - qlo
                        off = UOFF[m] if merged else 0
                        last = last_unit and m == unit[-1]
                        if m in MERGED:
                            vi = qb * 384 + MERGED[m] * 128
                            den_st = const_sb["valid"][:, vi:vi + 128]
                        else:
                            den_st = const_sb["ones"][:]
                        nc.tensor.matmul(
                            acc_y[:, qlo:qhi],
                            v_sb[4 * qb + m][:, gl * 128:(gl + 1) * 128],
                            p[:, off:off + w], start=first, stop=last)
                        nc.tensor.matmul(acc_d[:, qlo:qhi], den_st,
                                         p[:, off:off + w], start=first,
                                         stop=last)
                        first = False

                for i in range(LOOK):
                    qk(i)
                for i in range(len(UNITS)):
                    if i + LOOK < len(UNITS):
                        qk(i + LOOK)
                    pv(i)

                rcp = fin.tile([128, 512], F32, tag="rcp", name=f"rcp{vh}")
                nc.vector.reciprocal_approx_fast(rcp[:], acc_d[:])
                nc.vector.tensor_mul(
                    yt_sb[hl][:, qb * 512:(qb + 1) * 512], acc_y[:], rcp[:])

        # ---- phase C: partial O^T projection (local heads only; host sums
        # the two partials per q-range)
        for n0 in range(0, NCC, 2):
            np_ = n0 // 2
            wot = wop.tile([128, 2048], BF16, tag="wo", name=f"wot{np_}")
            nc.sync.dma_start(wot[:], wo[np_ * 128:(np_ + 1) * 128, :])
            for qb in range(QB):
                opair = [ps_acc.tile([128, 512], F32, tag="acc",
                                     name=f"ops{n0}_{qb}_{j}")
                         for j in range(2)]
                for hl in range(LH):
                    for j in range(2):
                        nc.tensor.matmul(
                            opair[j][:],
                            wot[:, hl * 256 + j * 128:hl * 256 + (j + 1) * 128],
                            yt_sb[hl][:, qb * 512:(qb + 1) * 512],
                            start=(hl == 0), stop=(hl == LH - 1))
                for j in range(2):
                    osb = fin.tile([128, 512], BF16, tag="osb",
                                   name=f"osb{n0}_{qb}_{j}")
                    nc.scalar.activation(osb[:], opair[j][:], Copy)
                    nc.sync.dma_start(
                        outT[(n0 + j) * 128:(n0 + j + 1) * 128,
                             qb * 512:(qb + 1) * 512], osb[:])


# ---------------------------------------------------------------- host side
def _host_inputs(x, Wq, Wk, Wv, Wo):
    x = np.asarray(x, dtype=np.float32).reshape(T, DIM)
    Wq = np.asarray(Wq, np.float32)
    Wk = np.asarray(Wk, np.float32)
    Wv = np.asarray(Wv, np.float32)
    Wo = np.asarray(Wo, np.float32)

    inv_freq = 1.0 / (ROPE_BASE ** (np.arange(0, D, 2, dtype=np.float64) / D))
    dfreq = np.concatenate([inv_freq, inv_freq])  # [128] per-dim freq

    u = np.arange(128)[:, None]
    maskB = (np.arange(128)[None, :] < u).astype(BF)        # qq>=u -> 0
    maskA = (u <= np.arange(128)[None, :]).astype(BF)       # u>qq -> 0

    rotp = np.zeros((128, 128), np.float32)
    d = np.arange(128)
    rotp[(d + 64) % 128, d] = 1.0  # out[d] = in[(d+64)%128]
    rotp = rotp.astype(BF)

    ones = np.ones((128, 128), BF)

    # per head-half hh: packed weight slices
    wq_hh, wk_hh, wv_hh, wo_hh = [], [], [], []
    for hh in range(2):
        wq_hh.append(np.ascontiguousarray(
            Wq[:, hh * 1024:(hh + 1) * 1024]
            .reshape(NCC, 128, 4, 256).transpose(2, 1, 0, 3)
            .reshape(4 * 128, 4096)).astype(BF))
        wk_hh.append(np.ascontiguousarray(
            Wk[:, hh * 256:(hh + 1) * 256]
            .reshape(NCC, 128, 256).transpose(1, 0, 2)
            .reshape(128, NCC * 256)).astype(BF))
        wv_hh.append(np.ascontiguousarray(
            Wv[:, hh * 256:(hh + 1) * 256]
            .reshape(NCC, 128, 256).transpose(1, 0, 2)
            .reshape(128, NCC * 256)).astype(BF))
        wo_hh.append(np.ascontiguousarray(
            Wo[hh * 1024:(hh + 1) * 1024, :]
            .reshape(8, 128, 8, 256).transpose(2, 1, 0, 3)
            .reshape(8 * 128, 2048)).astype(BF))

    in_maps = []
    for c in range(N_CORES):
        rc = c % SEQ_SH
        hh = c // SEQ_SH
        qs = rc * TQ
        xkv = np.zeros((TKV, DIM), np.float32)  # [2048, 2048]
        lo = qs - WIN
        src_lo = max(0, lo)
        xkv[src_lo - lo:TKV] = x[src_lo:qs + TQ]

        pos_q = np.arange(qs, qs + TQ, dtype=np.float64)
        pos_k = np.arange(lo, qs + TQ, dtype=np.float64)
        angq = dfreq[:, None] * pos_q[None, :]  # [128, 1024]
        angk = dfreq[:, None] * pos_k[None, :]  # [128, 2048]
        sgn = np.where(np.arange(D) < D // 2, -1.0, 1.0)[:, None]

        kb = np.zeros((128, QB * NMT), np.float32)
        vld = np.zeros((128, QB * 384), np.float32)
        for qb in range(QB):
            winq = WIN - qs - 512 * qb   # local slots below this are padding
            for m in range(NMT):
                t_loc = 128 * m + np.arange(128)
                kb[:, qb * NMT + m] = np.where(t_loc < winq, -30.0, 0.0)
            for vi, m in enumerate((0, 1, 2)):
                t_loc = 128 * m + np.arange(128)
                vld[:, qb * 384 + vi * 128:qb * 384 + (vi + 1) * 128] = \
                    np.where(t_loc < winq, 0.0, 1.0)[:, None]

        in_maps.append({
            "xkvT": np.ascontiguousarray(
                xkv.T.reshape(DIM, NSP, 512).transpose(1, 0, 2)
                .reshape(NSP * DIM, 512)).astype(BF),
            "wq": wq_hh[hh], "wk": wk_hh[hh], "wv": wv_hh[hh],
            "wo": wo_hh[hh],
            "cosq": np.cos(angq).astype(np.float32),
            "sinq": (sgn * np.sin(angq)).astype(np.float32),
            "cosk": np.ascontiguousarray(np.cos(angk).astype(np.float32)
                .reshape(D, NSP, 512).transpose(1, 0, 2))
                .reshape(NSP * D, 512),
            "sink": np.ascontiguousarray(((sgn * np.sin(angk))
                .astype(np.float32))
                .reshape(D, NSP, 512).transpose(1, 0, 2))
                .reshape(NSP * D, 512),
            "kbias": kb,
            "valid": vld.astype(BF),
            "maskB": maskB, "maskA": maskA,
            "rotp": rotp,
            "ones": ones,
        })
    return in_maps


def kernel(x, Wq, Wk, Wv, Wo, _trace=False, _trace_kwargs=None):
    nc = _build()
    in_maps = _host_inputs(x, Wq, Wk, Wv, Wo)
    res = run_bass_kernel_spmd(nc, in_maps, core_ids=list(range(N_CORES)),
                               trace=_trace, **(_trace_kwargs or {}))
    out = np.zeros((1, T, DIM), np.float32)
    for c in range(N_CORES):
        rc = c % SEQ_SH
        out[0, rc * TQ:(rc + 1) * TQ, :] += \
            res.results[c]["outT"].T.astype(np.float32)
    if _trace:
        kernel.last_results = res
    return out
